# revision 4
# baseline (speedup 1.0000x reference)
"""Trainium2 Bass kernel v3: MHA with sparsemax over the key dim.

Reference computation (B=2, S=2048, D=256, H=8, Dk=32):
    q = (query @ Wq.T + bq)  -> [B,S,H,Dk]   (k, v likewise)
    attn = einsum('bihd,bjhd->bijh', q, k) / sqrt(Dk)
    attn = sparsemax(attn, axis=-2)           # normalize over Sk (j) per (b,i,h)
    out  = einsum('bijh,bjhd->bihd', attn, v) -> reshape [B,S,256]

Sharding: 8 cores = 2 batches x 4 head-pairs. No collectives.

v3 design (vs v2): output NORMALIZATION replaces exact-tau convergence.
  The PV matmul's stationary v is augmented with a ones column, so PSUM row
  32 accumulates A_i = sum_j p[i,j] for free (matmul cost is moving-cols
  only). The host divides by A and adds the v-bias afterwards; sparsemax's
  sum-to-1 constraint is then exact regardless of tau error, which lets us
  drop the trapezoid pass, the submax probe/shift, and the fp16 tau hi/lo
  split (algo-sim rel err 4.9e-3 vs 2e-2 budget; work/algo_sim3.py).

  Pipeline per 128-row z tile:
    1. PE: z = qT.T @ kT (fp16) -> PSUM; ACT: z' = relu(z) fp16 -> SBUF.
    2. DVE: pairwise-max fold tree on z' -> 256 group maxes (l3) + rowmax.
    3. 3 damped Newton iters on the group-max surrogate from d0 = rowmax-1
       (Pool computes the relu(l3-d) scratch, DVE accumulates at 4x).
    4. 2 exact Newton steps on z': A via DVE 2-op (6 of 8 tiles) or ACT
       relu-bias-accum (2 of 8, rebalancing); C via DVE is_gt-accum 4x.
  p in column layout via a SECOND PE matmul zT = kTa.T @ qTa with one
  augmented contraction row (ones in kTa; -tau fp16 in qTa, filled per tile
  via a PE transpose). pT = relu(zT_psum) copies run on ACT (DVE gets no
  2x from an fp32-PSUM source, so ACT's 0.833 ns/col wins); PV is
  v-stationary with the ones column giving A. Output [97, S] d-major
  (head h at rows 64h: 32 pv rows + 1 A row); the host transposes and
  normalizes.

Engine balance (TimelineSim): ACT ~184us (z/pT copies, proj, 2/8 A-passes,
copy-out), DVE ~186us (fold, warm, A/C passes, tau, v), Pool ~45us (warm
scratch), PE ~93us -- ACT/DVE both ~84% busy over the 220us makespan.
Schedule: 8 groups of 4 tiles, wavefront stride 2; Newton A-scratches are
kept OFF Pool (its 1.39ns/col + per-op launch put it on the critical
chain); the last group's pT copies alternate ACT/DVE to share the drain.

Projection packing: qTa/kTa are [97, S] (head h at partition base 64h:
32 proj rows + 1 aug row -- matmul stationary bases must be 0/32/64), so
each 512-col projection chunk is ONE [97,512] PSUM->SBUF copy. The kTa
aug row of ones comes free from the bias K=1 matmul (bk_pad aug slot =
1.0). xq/xk DMAs are split in column halves so projections start early.

PE wait discipline: walrus allows only ONE sync wait on a PE LDWEIGHTS
struct, so every PE matmul/transpose is kept to at most one semaphore wait:
multi-source waits are absorbed by chains of tiny real LDWEIGHTS "fence"
instructions (one semaphore each), and PSUM slot reacquisition is gated on
the slot's previous reader (PsumTag).
"""

import numpy as np
from contextlib import ExitStack

HEADS = 8
D_MODEL = 256
DK = 32
B = 2
S = 2048
SCALE = float(1.0 / np.float32(np.sqrt(DK)))
N_CORES = 8
NT = 16            # i-tiles per head (2048/128)
NTILES = 32        # z tiles per core (2 heads x 16)
GROUP = 8          # tiles per group (half a head)
NGRP = NTILES // GROUP
NSEG = 256         # fold-tree level-3 groups per row
WARM_LAMBDAS = (1.5, 1.0, 1.0)
N_FULL = 2         # exact Newton steps
HR = 33            # rows per head in qTa/kTa (32 proj + 1 aug) and out

_PROGRAM = None


def _build_program(loop_n=1):
    import concourse.bass as bass
    import concourse.mybir as mybir
    import concourse.tile as tile
    from concourse import bacc
    from concourse.tile import add_dep_helper
    from concourse.masks import make_identity

    f32 = mybir.dt.float32
    f16 = mybir.dt.float16
    AX = mybir.AxisListType
    OP = mybir.AluOpType
    ACTF = mybir.ActivationFunctionType

    nc = bacc.Bacc("TRN2", target_bir_lowering=False, debug=False)

    # Per-core inputs (host pre-sliced / pre-transposed / fp16-cast).
    xqT_d = nc.dram_tensor("xqT", [D_MODEL, S], f16, kind="ExternalInput")
    xkT_d = nc.dram_tensor("xkT", [D_MODEL, S], f16, kind="ExternalInput")
    xvT_d = nc.dram_tensor("xvT", [D_MODEL, S], f16, kind="ExternalInput")
    wqT_d = nc.dram_tensor("wqT", [D_MODEL, 2 * HR], f16, kind="ExternalInput")
    wkT_d = nc.dram_tensor("wkT", [D_MODEL, 2 * HR], f16, kind="ExternalInput")
    wvT_d = nc.dram_tensor("wvT", [D_MODEL, 64], f16, kind="ExternalInput")
    bq_d = nc.dram_tensor("bq", [1, 2 * HR], f16, kind="ExternalInput")
    bk_d = nc.dram_tensor("bk", [1, 2 * HR], f16, kind="ExternalInput")
    out_d = nc.dram_tensor("out", [2 * HR, S], f32, kind="ExternalOutput")

    import concourse.mybir as _mb

    gate_state = {"w": None}

    def pe_fence(dep_instrs):
        """Chain of tiny real PE LDWEIGHTS instructions that absorb waits so
        later PE matmuls carry at most one semaphore wait each."""
        groups = {}
        for d in dep_instrs:
            if d is None:
                continue
            eng = d.ins.engine
            key = ("dma", d.ins.name) if eng == _mb.EngineType.SP else eng
            groups.setdefault(key, []).append(d)
        last = None
        for key, ds in groups.items():
            g = nc.tensor.ldweights(weights=gate_state["w"][:1, :1])
            for d in ds:
                add_dep_helper(g.ins, d.ins, sync=True, reason="pe-fence")
            if last is not None:
                add_dep_helper(g.ins, last.ins, sync=False, reason="pe-fence-chain")
            last = g
        return last

    class PsumTag:
        """Psum slot allocator wrapper that gates each slot's reacquisition
        on its previous reader via a PE fence (keeps matmul waits <= 1)."""

        def __init__(self, pool, shape, dtype, tag, bufs):
            self.pool, self.shape, self.dtype, self.tag = pool, shape, dtype, tag
            self.bufs = bufs
            self.hist = [None] * bufs
            self.i = 0

        def tile(self, shape=None, extra_deps=(), dtype=None):
            k = self.i % self.bufs
            self.i += 1
            deps = list(extra_deps)
            if self.hist[k]:
                deps.extend(self.hist[k])
            gate = pe_fence(deps) if deps else None
            t = self.pool.tile(shape or self.shape, dtype or self.dtype,
                               tag=self.tag)
            return t, gate, k

        def readers(self, k, instrs):
            self.hist[k] = [i for i in instrs if i is not None]

    with tile.TileContext(nc) as tc, ExitStack() as ctx:
        singles = ctx.enter_context(tc.tile_pool(name="singles", bufs=1))
        psum = ctx.enter_context(tc.tile_pool(name="psum", bufs=2, space="PSUM"))
        ztpsum = ctx.enter_context(tc.tile_pool(name="ztpsum", bufs=2, space="PSUM"))
        pvpsum = ctx.enter_context(tc.tile_pool(name="pvpsum", bufs=2, space="PSUM"))

        # ---- constants / small persistent buffers ----
        gate_w = singles.tile([1, 8], f16)
        i_gw = nc.vector.memset(gate_w, 0.0)
        gate_state["w"] = gate_w
        # preload the Relu activation table during the DMA/proj prologue
        nc.scalar.activation(gate_w[:], gate_w[:], mybir.ActivationFunctionType.Relu)
        ident = singles.tile([128, 128], f16)
        nc.gpsimd.memset(ident, 0.0)
        i_ident = nc.gpsimd.affine_select(
            out=ident, in_=ident, compare_op=mybir.AluOpType.not_equal,
            fill=1.0, base=0, pattern=[[-1, 128]], channel_multiplier=1)

        bq_sb = singles.tile([1, 2 * HR], f16)
        bk_sb = singles.tile([1, 2 * HR], f16)
        ones_row = singles.tile([1, 512], f16)
        i_ones = nc.vector.memset(ones_row, 1.0)
        i_bq = nc.sync.dma_start(bq_sb[:], bq_d[:])
        i_bk = nc.sync.dma_start(bk_sb[:], bk_d[:])

        # per-head packed projections: per head 32 proj rows + 1 aug row
        qTa = singles.tile([2 * HR, S], f16)
        kTa = singles.tile([2 * HR, S], f16)
        v_sb = singles.tile([128, NT, 2, HR], f16)  # [j%128, j//128, h, d+ones]
        i_vones = nc.vector.memset(v_sb[:, :, :, 32], 1.0)
        outT_sb = singles.tile([2 * HR, S], f32)    # [h*(33)+d | A, i]

        # per-tile stat columns [128, NTILES]
        mrow = singles.tile([128, NTILES], f32)     # rowmax of z'
        dbuf = singles.tile([128, NTILES], f32)     # tau estimate
        ndbuf = singles.tile([128, NTILES], f32)    # -dbuf
        Abuf = singles.tile([128, NTILES], f32)
        Cbuf = singles.tile([128, NTILES], f32)
        rcb = singles.tile([128, NTILES], f32)
        stb = singles.tile([128, NTILES], f32)
        nthi = singles.tile([128, NTILES], f16)     # fp16(-tau)

        # scratch (single-buffered; same-engine ops serialize in order)
        act_scr = singles.tile([128, S], f16)
        dve_scr = singles.tile([128, S], f16)
        segC_scr = singles.tile([128, NSEG], f16)
        segA_scr = singles.tile([128, GROUP, NSEG], f16)
        segA_acc = singles.tile([128, GROUP, NSEG], f16)
        mrow_scr = singles.tile([128, NSEG], f16)

        zps_slots = PsumTag(psum, [128, 1024], f32, "zps", 2)

        def emit_proj(xpool):
            xq = xpool.tile([128, 2, S], f16, tag="xq")
            xk = xpool.tile([128, 2, S], f16, tag="xk")
            xv = xpool.tile([128, 2, S], f16, tag="xv")
            wq = xpool.tile([128, 2, 2 * HR], f16, tag="wq")
            wk = xpool.tile([128, 2, 2 * HR], f16, tag="wk")
            wv = xpool.tile([128, 2, 64], f16, tag="wv")
            d_wk = nc.sync.dma_start(wk[:], wkT_d[:].rearrange("(c p) d -> p c d", p=128))
            d_wq = nc.sync.dma_start(wq[:], wqT_d[:].rearrange("(c p) d -> p c d", p=128))
            xk_r = xkT_d[:].rearrange("(c p) i -> p c i", p=128)
            xq_r = xqT_d[:].rearrange("(c p) i -> p c i", p=128)
            d_xk, d_xq = [], []
            for n in (0, 1):
                sl = slice(n * 1024, (n + 1) * 1024)
                d_xk.append(nc.sync.dma_start(xk[:, :, sl], xk_r[:, :, sl]))
                d_xq.append(nc.sync.dma_start(xq[:, :, sl], xq_r[:, :, sl]))
            d_wv = nc.sync.dma_start(wv[:], wvT_d[:].rearrange("(c p) d -> p c d", p=128))
            d_xv = nc.sync.dma_start(xv[:], xvT_d[:].rearrange("(c p) i -> p c i", p=128))
            gates = {"k0": pe_fence([i_gw, d_wk, d_xk[0], i_bk, i_ones]),
                     "k1": pe_fence([d_xk[1]]),
                     "q0": pe_fence([d_wq, d_xq[0], i_bq]),
                     "q1": pe_fence([d_xq[1]]),
                     "v": pe_fence([d_wv, d_xv, i_vones])}

            qk_eps, v_eps = [], []
            # qT/kT: psum [66, 512] = W.T @ x per 512-col chunk, both heads
            # packed; aug rows come from the bias matmul (bk aug slot = 1).
            def emit_chunk(which, w, x, bias, dst, n):
                ps, gq, kq = zps_slots.tile([2 * HR, 512])
                sl = slice(n * 512, (n + 1) * 512)
                for c in range(2):
                    mm = nc.tensor.matmul(ps[:], w[:, c, :], x[:, c, sl],
                                          start=(c == 0), stop=False)
                    add_dep_helper(mm.ins, (gq or gates[which]).ins,
                                   sync=False, reason="ord")
                mm = nc.tensor.matmul(ps[:], bias[:], ones_row[:],
                                      start=False, stop=True)
                add_dep_helper(mm.ins, (gq or gates[which]).ins,
                               sync=False, reason="ord")
                e = nc.scalar.copy(dst[:, sl], ps[:])
                zps_slots.readers(kq, [e])
                return e

            for n in range(4):
                qk_eps.append(emit_chunk("k", wk, xk, bk_sb, kTa, n))
            q_eps_by_chunk = {0: emit_chunk("q", wq, xq, bq_sb, qTa, 0),
                              1: emit_chunk("q", wq, xq, bq_sb, qTa, 1)}

            def emit_late_q():
                q_eps_by_chunk[2] = emit_chunk("q", wq, xq, bq_sb, qTa, 2)
                q_eps_by_chunk[3] = emit_chunk("q", wq, xq, bq_sb, qTa, 3)

            # v[j, d] = x @ Wv.T (no bias; host adds it after normalize) --
            # deferred: the caller emits it off the z-matmul critical path,
            # on the ztp psum slots (idle until the first P stage)
            def emit_v(ztp_slots):
                for jt in range(NT):
                    ps_v, gv, kv = ztp_slots.tile([128, 64])
                    jsl = slice(jt * 128, (jt + 1) * 128)
                    for c in range(2):
                        mm = nc.tensor.matmul(ps_v[:], xv[:, c, jsl],
                                              wv[:, c, :],
                                              start=(c == 0), stop=(c == 1))
                        add_dep_helper(mm.ins, (gv or gates["v"]).ins,
                                       sync=False, reason="ord")
                    ev = nc.vector.tensor_copy(v_sb[:, jt, :, 0:32], ps_v[:])
                    ztp_slots.readers(kv, [ev])
                    v_eps.append(ev)
            return qk_eps, q_eps_by_chunk, emit_late_q, v_eps, emit_v

        def emit_main(k_eps, q_eps_by_chunk, emit_late_q, v_eps, emit_v):
            v_gate_deps = v_eps  # filled by emit_v, consumed by first stage_P
            once_deps = [i_ident]
            kgates, qgates = {}, {}

            def kgate_for(c):
                if c not in kgates:
                    kgates[c] = pe_fence([k_eps[c]])
                return kgates[c]

            def qgate_for(qc):
                if qc not in qgates:
                    qgates[qc] = pe_fence([q_eps_by_chunk[qc]])
                return qgates[qc]
            ztp_slots = PsumTag(ztpsum, [128, 512], f32, "ztp", 2)
            pv_slots = PsumTag(pvpsum, [HR, 512], f32, "pv", 2)

            zp_tiles = {}

            def tile_hd(t):
                return t // NT, t % NT  # head, i-tile

            def batched_update(gsl, lam):
                """dbuf += lam * (Abuf - 1) / Cbuf; ndbuf = -dbuf.
                C >= 1 is guaranteed while d < rowmax' (the max element always
                counts)."""
                nc.vector.reciprocal(rcb[:, gsl], Cbuf[:, gsl])
                nc.vector.scalar_tensor_tensor(stb[:, gsl], Abuf[:, gsl], -1.0,
                                               rcb[:, gsl], OP.add, OP.mult)
                nc.vector.scalar_tensor_tensor(dbuf[:, gsl], stb[:, gsl], lam,
                                               dbuf[:, gsl], OP.mult, OP.add)
                nc.vector.tensor_scalar(ndbuf[:, gsl], dbuf[:, gsl], -1.0,
                                        None, OP.mult)

            def stage_Z(grp):
                """z matmuls + relu copy to SBUF fp16 (ACT)."""
                g0 = grp * GROUP
                h = g0 // NT
                r0 = h * HR
                for t in range(g0, g0 + GROUP):
                    _, it = tile_hd(t)
                    isl = slice(it * 128, (it + 1) * 128)
                    zp = zpool.tile([128, S], f16, tag="zp")
                    for n in range(2):
                        zps, gz, kz = zps_slots.tile()
                        for m in range(2):
                            nsl = slice((2 * n + m) * 512, (2 * n + m + 1) * 512)
                            mm = nc.tensor.matmul(
                                zps[:, m * 512:(m + 1) * 512],
                                qTa[r0:r0 + 32, isl], kTa[r0:r0 + 32, nsl],
                                start=True, stop=True)
                            for dep in (gz, kgate_for(2 * n + m),
                                        qgate_for(it // 4)):
                                if dep is not None:
                                    add_dep_helper(mm.ins, dep.ins,
                                                   sync=False, reason="ord")
                        bsl = slice(n * 1024, (n + 1) * 1024)
                        cp = nc.scalar.activation(zp[:, bsl], zps[:], ACTF.Relu)
                        zps_slots.readers(kz, [cp])
                    zp_tiles[t] = zp

            def stage_T(grp):
                """fold tree to 256 strided groups + rowmax (DVE, fp16 2x)."""
                g0 = grp * GROUP
                for t in range(g0, g0 + GROUP):
                    zp = zp_tiles[t]
                    l1 = trpool.tile([128, 1024], f16, tag="l1")
                    l2 = trpool.tile([128, 512], f16, tag="l2")
                    l3 = l3pool.tile([128, NSEG], f16, tag="l3")
                    nc.vector.tensor_tensor(l1[:], zp[:, 0:1024],
                                            zp[:, 1024:2048], OP.max)
                    nc.vector.tensor_tensor(l2[:], l1[:, 0:512],
                                            l1[:, 512:1024], OP.max)
                    nc.vector.tensor_tensor(l3[:], l2[:, 0:256],
                                            l2[:, 256:512], OP.max)
                    nc.vector.tensor_scalar(
                        mrow_scr[:], l3[:], 0.0, None, OP.add, OP.max,
                        accum_out=mrow[:, t:t + 1])
                    zp_tiles[t] = (zp, l3)

            def stage_N_warm(grp, k):
                """one damped Newton iteration on group maxes (Pool+DVE)."""
                g0 = grp * GROUP
                gsl = slice(g0, g0 + GROUP)
                if k == 0:
                    # d0 = rowmax' - 1
                    nc.vector.tensor_scalar(dbuf[:, gsl], mrow[:, gsl], 1.0,
                                            None, OP.subtract)
                    nc.vector.tensor_scalar(ndbuf[:, gsl], mrow[:, gsl], -1.0,
                                            1.0, OP.mult, OP.add)
                for t in range(g0, g0 + GROUP, 2):
                    _, l3 = zp_tiles[t]
                    sl = t - g0
                    nc.gpsimd.tensor_scalar(
                        segA_scr[:, sl, :], l3[:], dbuf[:, t:t + 1], 0.0,
                        OP.subtract, OP.max)
                for t in range(g0, g0 + GROUP):
                    sl = t - g0
                    _, l3 = zp_tiles[t]
                    if t % 2 == 0:
                        nc.vector.tensor_scalar(
                            segA_acc[:, sl, :], segA_scr[:, sl, :], 0.0, None,
                            OP.add, OP.add, accum_out=Abuf[:, t:t + 1])
                    else:
                        nc.vector.tensor_scalar(
                            segA_acc[:, sl, :], l3[:], dbuf[:, t:t + 1], 0.0,
                            OP.subtract, OP.max)
                        nc.vector.tensor_scalar(
                            segA_acc[:, sl, :], segA_acc[:, sl, :], 0.0, None,
                            OP.add, OP.add, accum_out=Abuf[:, t:t + 1])
                    nc.vector.tensor_scalar(
                        segC_scr[:], l3[:], dbuf[:, t:t + 1], None,
                        OP.is_gt, OP.add, accum_out=Cbuf[:, t:t + 1])
                batched_update(gsl, WARM_LAMBDAS[k])

            def stage_N_newton(grp, it_n):
                """exact Newton: A on ACT (step 0) / DVE 2-op (step 1);
                C on DVE 4x."""
                g0 = grp * GROUP
                gsl = slice(g0, g0 + GROUP)
                for t in range(g0, g0 + GROUP):
                    zp, _ = zp_tiles[t]
                    if it_n == 0:
                        nc.scalar.activation(act_scr[:], zp[:], ACTF.Relu,
                                             bias=ndbuf[:, t:t + 1], scale=1.0,
                                             accum_out=Abuf[:, t:t + 1])
                    else:
                        # DVE 2-op form (both at 4x): relu scratch, then sum
                        nc.vector.tensor_scalar(
                            dve_scr[:], zp[:], dbuf[:, t:t + 1], 0.0,
                            OP.subtract, OP.max)
                        nc.vector.tensor_scalar(
                            dve_scr[:], dve_scr[:], 0.0, None,
                            OP.add, OP.add, accum_out=Abuf[:, t:t + 1])
                    nc.vector.tensor_scalar(dve_scr[:], zp[:],
                                            dbuf[:, t:t + 1], None,
                                            OP.is_gt, OP.add,
                                            accum_out=Cbuf[:, t:t + 1])
                batched_update(gsl, 1.0)
                if it_n == N_FULL - 1:
                    for t in range(g0, g0 + GROUP):
                        zp_tiles.pop(t)

            def stage_N_tau(grp):
                """finalize tau: fp16 cast, aug-row fill via PE transpose."""
                g0 = grp * GROUP
                gsl = slice(g0, g0 + GROUP)
                h = g0 // NT
                r0 = h * HR
                e_hi = nc.gpsimd.tensor_copy(nthi[:, gsl], ndbuf[:, gsl])

                # per tile: PE transpose [128,1] -> [1,128] -> qTa aug row
                pgate = pe_fence([e_hi] + once_deps)
                once_deps.clear()
                tau_eps = []
                for t in range(g0, g0 + GROUP):
                    _, it = tile_hd(t)
                    isl = slice(it * 128, (it + 1) * 128)
                    tps, gt, kt = ztp_slots.tile([1, 128], dtype=f16)
                    tr = nc.tensor.transpose(tps[:], nthi[:, t:t + 1], ident[:])
                    add_dep_helper(tr.ins, (gt or pgate).ins, sync=False,
                                   reason="ord")
                    ct = nc.vector.tensor_copy(qTa[r0 + 32:r0 + 33, isl], tps[:])
                    ztp_slots.readers(kt, [ct])
                    tau_eps.append(ct)
                return tau_eps

            def stage_P(grp, tau_eps):
                """zT (tau-shifted) + pT relu-copy + PV(+A) + copy-out."""
                g0 = grp * GROUP
                h = g0 // NT
                r0 = h * HR
                i0 = (g0 % NT) * 128
                NCK = GROUP * 128 // 512
                zgate = pe_fence(tau_eps + v_gate_deps)
                v_gate_deps.clear()
                pvs = [pv_slots.tile() for _ in range(NCK)]
                alt = 0
                # late groups: alternate zT chunks onto the (now idle) z-matmul
                # PSUM banks for a 4-deep copy pipeline in the drain phase
                borrow = grp >= NGRP - 2
                for jb in range(NT):
                    pT = ptpool.tile([128, GROUP * 128], f16, tag="pT")
                    jsl = slice(jb * 128, (jb + 1) * 128)
                    pcs = []
                    for cnk in range(NCK):
                        if borrow and (jb * NCK + cnk) % 2 == 1:
                            ztps, gzt, kzt0 = zps_slots.tile([128, 512])
                            kzt = ("z", kzt0)
                        else:
                            ztps, gzt, kzt0 = ztp_slots.tile()
                            kzt = ("t", kzt0)
                        csl = slice(i0 + cnk * 512, i0 + (cnk + 1) * 512)
                        mm = nc.tensor.matmul(ztps[:], kTa[r0:r0 + HR, jsl],
                                              qTa[r0:r0 + HR, csl],
                                              start=True, stop=True)
                        add_dep_helper(mm.ins, (gzt or zgate).ins, sync=False,
                                       reason="ord")
                        psl = slice(cnk * 512, (cnk + 1) * 512)
                        if alt % 4 < 3:
                            pc = nc.gpsimd.tensor_scalar(pT[:, psl], ztps[:],
                                                         0.0, None, OP.max)
                        else:
                            pc = nc.vector.tensor_scalar(pT[:, psl], ztps[:],
                                                         0.0, None, OP.max)
                        alt += 1
                        (ztp_slots if kzt[0] == "t" else zps_slots).readers(
                            kzt[1], [pc])
                        pcs.append(pc)
                    pgate2 = pe_fence(pcs)
                    for cnk in range(NCK):
                        pv_t, pv_g, pv_k = pvs[cnk]
                        mm = nc.tensor.matmul(pv_t[:], v_sb[:, jb, h, :],
                                              pT[:, cnk * 512:(cnk + 1) * 512],
                                              start=(jb == 0),
                                              stop=(jb == NT - 1))
                        add_dep_helper(mm.ins, pgate2.ins, sync=False,
                                       reason="ord")
                        if jb == 0 and pv_g is not None:
                            add_dep_helper(mm.ins, pv_g.ins, sync=False,
                                           reason="pv-slot")

                # copy-out (plain; host normalizes by the A row + adds bias)
                for cnk in range(NCK):
                    pv_t, pv_g, pv_k = pvs[cnk]
                    c0 = i0 + cnk * 512
                    oc = nc.gpsimd.tensor_copy(
                        outT_sb[r0:r0 + HR, c0:c0 + 512], pv_t[:])
                    pv_slots.readers(pv_k, [oc])

            # wavefront schedule: per-group step chains staggered by STRIDE
            # rows so engines always have ready work
            taus = {}
            gsteps = (["Z", "T"] + [f"w{k}" for k in range(len(WARM_LAMBDAS))]
                      + [f"n{k}" for k in range(N_FULL)] + ["tau", "P"])
            STRIDE = 3
            schedule = []
            nrows = (NGRP - 1) * STRIDE + len(gsteps)
            for r in range(nrows):
                for g in range(NGRP):
                    k = r - g * STRIDE
                    if 0 <= k < len(gsteps):
                        schedule.append((gsteps[k], g))
            emit_late_q()
            v_emitted = False
            for op, g in schedule:
                if op == "Z":
                    stage_Z(g)
                    if not v_emitted:
                        emit_v(ztp_slots)
                        v_emitted = True
                elif op == "T":
                    stage_T(g)
                elif op.startswith("w"):
                    stage_N_warm(g, int(op[1]))
                elif op.startswith("n"):
                    stage_N_newton(g, int(op[1]))
                elif op == "tau":
                    taus[g] = stage_N_tau(g)
                elif op == "P":
                    stage_P(g, taus.pop(g))

            nc.sync.dma_start(out_d[:], outT_sb[:])

        if loop_n > 1:
            xpool = ctx.enter_context(tc.tile_pool(name="xstage", bufs=1))
            zpool = ctx.enter_context(tc.tile_pool(name="zpool", bufs=24))
            trpool = ctx.enter_context(tc.tile_pool(name="trpool", bufs=2))
            l3pool = ctx.enter_context(tc.tile_pool(name="l3pool", bufs=24))
            ptpool = ctx.enter_context(tc.tile_pool(name="ptpool", bufs=6))
            with tc.For_i(0, loop_n, 1):
                emit_main(*emit_proj(xpool))
        else:
            xpool = ctx.enter_context(tc.tile_pool(name="xstage", bufs=1))
            proj_eps = emit_proj(xpool)
            zpool = ctx.enter_context(tc.tile_pool(name="zpool", bufs=24))
            trpool = ctx.enter_context(tc.tile_pool(name="trpool", bufs=2))
            l3pool = ctx.enter_context(tc.tile_pool(name="l3pool", bufs=24))
            ptpool = ctx.enter_context(tc.tile_pool(name="ptpool", bufs=6))
            emit_main(*proj_eps)

    nc.finalize()
    return nc


def _get_program(loop_n=1):
    global _PROGRAM
    if _PROGRAM is None:
        _PROGRAM = {}
    if loop_n not in _PROGRAM:
        _PROGRAM[loop_n] = _build_program(loop_n)
    return _PROGRAM[loop_n]


def _pad_w(w):
    """[64, 256] head-pair weight block -> padded [256, 66] fp16 (aug cols
    32/65 zero)."""
    out = np.zeros((D_MODEL, 2 * HR), np.float16)
    out[:, 0:32] = w[0:32].T.astype(np.float16)
    out[:, HR:HR + 32] = w[32:64].T.astype(np.float16)
    return np.ascontiguousarray(out)


def _pad_b(b, aug):
    out = np.zeros((1, 2 * HR), np.float16)
    out[0, 0:32] = b[0:32].astype(np.float16)
    out[0, HR:HR + 32] = b[32:64].astype(np.float16)
    out[0, 32] = aug
    out[0, HR + 32] = aug
    return np.ascontiguousarray(out)


def _make_in_maps(query, key, value, Wq, bq, Wk, bk, Wv, bv):
    """Host-side sharding: slicing/transposition + fp16 cast of x and W."""
    f, g = np.float32, np.float16
    xq = [np.ascontiguousarray(np.asarray(query, f)[b].T.astype(g)) for b in range(B)]
    xk = [np.ascontiguousarray(np.asarray(key, f)[b].T.astype(g)) for b in range(B)]
    xv = [np.ascontiguousarray(np.asarray(value, f)[b].T.astype(g)) for b in range(B)]
    in_maps = []
    for c in range(N_CORES):
        b, gidx = c // 4, c % 4
        dsl = slice(gidx * 64, (gidx + 1) * 64)
        in_maps.append({
            "xqT": xq[b],
            "xkT": xk[b],
            "xvT": xv[b],
            "wqT": _pad_w(np.asarray(Wq, f)[dsl] * SCALE),
            "wkT": _pad_w(np.asarray(Wk, f)[dsl]),
            "wvT": np.ascontiguousarray(np.asarray(Wv, f)[dsl].T.astype(g)),
            "bq": _pad_b(np.asarray(bq, f)[dsl] * SCALE, 0.0),
            "bk": _pad_b(np.asarray(bk, f)[dsl], 1.0),
        })
    return in_maps


def kernel(query, key, value, Wq, bq, Wk, bk, Wv, bv):
    from concourse.bass_utils import run_bass_kernel_spmd

    nc = _get_program()
    in_maps = _make_in_maps(query, key, value, Wq, bq, Wk, bk, Wv, bv)
    res = run_bass_kernel_spmd(nc, in_maps, list(range(N_CORES)))
    bv32 = np.asarray(bv, np.float32)
    out = np.empty((B, S, D_MODEL), np.float32)
    for c in range(N_CORES):
        b, gidx = c // 4, c % 4
        r = res.results[c]["out"]                     # [66, S]
        for h in range(2):
            pv = r[h * HR:h * HR + 32]                # [32, S]
            A = np.maximum(r[h * HR + 32], 1e-3)      # [S]
            cols = slice(gidx * 64 + h * 32, gidx * 64 + (h + 1) * 32)
            out[b, :, cols] = (pv / A[None, :]).T + bv32[cols][None, :]
    return out


# revision 5
# speedup vs baseline: 1.0056x; 1.0056x over previous
"""Trainium2 Bass kernel v3: MHA with sparsemax over the key dim.

Reference computation (B=2, S=2048, D=256, H=8, Dk=32):
    q = (query @ Wq.T + bq)  -> [B,S,H,Dk]   (k, v likewise)
    attn = einsum('bihd,bjhd->bijh', q, k) / sqrt(Dk)
    attn = sparsemax(attn, axis=-2)           # normalize over Sk (j) per (b,i,h)
    out  = einsum('bijh,bjhd->bihd', attn, v) -> reshape [B,S,256]

Sharding: 8 cores = 2 batches x 4 head-pairs. No collectives.

v3 design (vs v2): output NORMALIZATION replaces exact-tau convergence.
  The PV matmul's stationary v is augmented with a ones column, so PSUM row
  32 accumulates A_i = sum_j p[i,j] for free (matmul cost is moving-cols
  only). The host divides by A and adds the v-bias afterwards; sparsemax's
  sum-to-1 constraint is then exact regardless of tau error, which lets us
  drop the trapezoid pass, the submax probe/shift, and the fp16 tau hi/lo
  split (algo-sim rel err 4.9e-3 vs 2e-2 budget; work/algo_sim3.py).

  Pipeline per 128-row z tile:
    1. PE: z = qT.T @ kT (fp16) -> PSUM; ACT: z' = relu(z) fp16 -> SBUF.
    2. DVE: pairwise-max fold tree on z' -> 256 group maxes (l3) + rowmax.
    3. 3 damped Newton iters on the group-max surrogate from d0 = rowmax-1
       (Pool computes the relu(l3-d) scratch, DVE accumulates at 4x).
    4. 2 exact Newton steps on z': A via DVE 2-op (6 of 8 tiles) or ACT
       relu-bias-accum (2 of 8, rebalancing); C via DVE is_gt-accum 4x.
  p in column layout via a SECOND PE matmul zT = kTa.T @ qTa with one
  augmented contraction row (ones in kTa; -tau fp16 in qTa, filled per tile
  via a PE transpose). pT = relu(zT_psum) copies run on ACT (DVE gets no
  2x from an fp32-PSUM source, so ACT's 0.833 ns/col wins); PV is
  v-stationary with the ones column giving A. Output [97, S] d-major
  (head h at rows 64h: 32 pv rows + 1 A row); the host transposes and
  normalizes.

Engine balance (TimelineSim): ACT ~184us (z/pT copies, proj, 2/8 A-passes,
copy-out), DVE ~186us (fold, warm, A/C passes, tau, v), Pool ~45us (warm
scratch), PE ~93us -- ACT/DVE both ~84% busy over the 220us makespan.
Schedule: 8 groups of 4 tiles, wavefront stride 2; Newton A-scratches are
kept OFF Pool (its 1.39ns/col + per-op launch put it on the critical
chain); the last group's pT copies alternate ACT/DVE to share the drain.

Projection packing: qTa/kTa are [97, S] (head h at partition base 64h:
32 proj rows + 1 aug row -- matmul stationary bases must be 0/32/64), so
each 512-col projection chunk is ONE [97,512] PSUM->SBUF copy. The kTa
aug row of ones comes free from the bias K=1 matmul (bk_pad aug slot =
1.0). xq/xk DMAs are split in column halves so projections start early.

PE wait discipline: walrus allows only ONE sync wait on a PE LDWEIGHTS
struct, so every PE matmul/transpose is kept to at most one semaphore wait:
multi-source waits are absorbed by chains of tiny real LDWEIGHTS "fence"
instructions (one semaphore each), and PSUM slot reacquisition is gated on
the slot's previous reader (PsumTag).
"""

import numpy as np
from contextlib import ExitStack

HEADS = 8
D_MODEL = 256
DK = 32
B = 2
S = 2048
SCALE = float(1.0 / np.float32(np.sqrt(DK)))
N_CORES = 8
NT = 16            # i-tiles per head (2048/128)
NTILES = 32        # z tiles per core (2 heads x 16)
GROUP = 8          # tiles per group (half a head)
NGRP = NTILES // GROUP
NSEG = 256         # fold-tree level-3 groups per row
WARM_LAMBDAS = (1.5, 1.0, 1.0)
N_FULL = 2         # exact Newton steps
HR = 33            # rows per head in qTa/kTa (32 proj + 1 aug) and out

_PROGRAM = None


def _build_program(loop_n=1):
    import concourse.bass as bass
    import concourse.mybir as mybir
    import concourse.tile as tile
    from concourse import bacc
    from concourse.tile import add_dep_helper
    from concourse.masks import make_identity

    f32 = mybir.dt.float32
    f16 = mybir.dt.float16
    AX = mybir.AxisListType
    OP = mybir.AluOpType
    ACTF = mybir.ActivationFunctionType

    nc = bacc.Bacc("TRN2", target_bir_lowering=False, debug=False)

    # Per-core inputs (host pre-sliced / pre-transposed / fp16-cast).
    xqT_d = nc.dram_tensor("xqT", [D_MODEL, S], f16, kind="ExternalInput")
    xkT_d = nc.dram_tensor("xkT", [D_MODEL, S], f16, kind="ExternalInput")
    xvT_d = nc.dram_tensor("xvT", [D_MODEL, S], f16, kind="ExternalInput")
    wqT_d = nc.dram_tensor("wqT", [D_MODEL, 2 * HR], f16, kind="ExternalInput")
    wkT_d = nc.dram_tensor("wkT", [D_MODEL, 2 * HR], f16, kind="ExternalInput")
    wvT_d = nc.dram_tensor("wvT", [D_MODEL, 64], f16, kind="ExternalInput")
    bq_d = nc.dram_tensor("bq", [1, 2 * HR], f16, kind="ExternalInput")
    bk_d = nc.dram_tensor("bk", [1, 2 * HR], f16, kind="ExternalInput")
    out_d = nc.dram_tensor("out", [2 * HR, S], f32, kind="ExternalOutput")

    import concourse.mybir as _mb

    gate_state = {"w": None}

    def pe_fence(dep_instrs):
        """Chain of tiny real PE LDWEIGHTS instructions that absorb waits so
        later PE matmuls carry at most one semaphore wait each."""
        groups = {}
        for d in dep_instrs:
            if d is None:
                continue
            eng = d.ins.engine
            key = ("dma", d.ins.name) if eng == _mb.EngineType.SP else eng
            groups.setdefault(key, []).append(d)
        last = None
        for key, ds in groups.items():
            g = nc.tensor.ldweights(weights=gate_state["w"][:1, :1])
            for d in ds:
                add_dep_helper(g.ins, d.ins, sync=True, reason="pe-fence")
            if last is not None:
                add_dep_helper(g.ins, last.ins, sync=False, reason="pe-fence-chain")
            last = g
        return last

    class PsumTag:
        """Psum slot allocator wrapper that gates each slot's reacquisition
        on its previous reader via a PE fence (keeps matmul waits <= 1)."""

        def __init__(self, pool, shape, dtype, tag, bufs):
            self.pool, self.shape, self.dtype, self.tag = pool, shape, dtype, tag
            self.bufs = bufs
            self.hist = [None] * bufs
            self.i = 0

        def tile(self, shape=None, extra_deps=(), dtype=None):
            k = self.i % self.bufs
            self.i += 1
            deps = list(extra_deps)
            if self.hist[k]:
                deps.extend(self.hist[k])
            gate = pe_fence(deps) if deps else None
            t = self.pool.tile(shape or self.shape, dtype or self.dtype,
                               tag=self.tag)
            return t, gate, k

        def readers(self, k, instrs):
            self.hist[k] = [i for i in instrs if i is not None]

    with tile.TileContext(nc) as tc, ExitStack() as ctx:
        singles = ctx.enter_context(tc.tile_pool(name="singles", bufs=1))
        psum = ctx.enter_context(tc.tile_pool(name="psum", bufs=2, space="PSUM"))
        ztpsum = ctx.enter_context(tc.tile_pool(name="ztpsum", bufs=2, space="PSUM"))
        pvpsum = ctx.enter_context(tc.tile_pool(name="pvpsum", bufs=2, space="PSUM"))

        # ---- constants / small persistent buffers ----
        gate_w = singles.tile([1, 8], f16)
        i_gw = nc.vector.memset(gate_w, 0.0)
        gate_state["w"] = gate_w
        # preload the Relu activation table during the DMA/proj prologue
        nc.scalar.activation(gate_w[:], gate_w[:], mybir.ActivationFunctionType.Relu)
        ident = singles.tile([128, 128], f16)
        nc.gpsimd.memset(ident, 0.0)
        i_ident = nc.gpsimd.affine_select(
            out=ident, in_=ident, compare_op=mybir.AluOpType.not_equal,
            fill=1.0, base=0, pattern=[[-1, 128]], channel_multiplier=1)

        bq_sb = singles.tile([1, 2 * HR], f16)
        bk_sb = singles.tile([1, 2 * HR], f16)
        ones_row = singles.tile([1, 512], f16)
        i_ones = nc.vector.memset(ones_row, 1.0)
        i_bq = nc.sync.dma_start(bq_sb[:], bq_d[:])
        i_bk = nc.sync.dma_start(bk_sb[:], bk_d[:])

        # per-head packed projections: per head 32 proj rows + 1 aug row
        qTa = singles.tile([2 * HR, S], f16)
        kTa = singles.tile([2 * HR, S], f16)
        v_sb = singles.tile([128, NT, 2, HR], f16)  # [j%128, j//128, h, d+ones]
        i_vones = nc.vector.memset(v_sb[:, :, :, 32], 1.0)
        outT_sb = singles.tile([2 * HR, S], f32)    # [h*(33)+d | A, i]

        # per-tile stat columns [128, NTILES]
        mrow = singles.tile([128, NTILES], f32)     # rowmax of z'
        dbuf = singles.tile([128, NTILES], f32)     # tau estimate
        ndbuf = singles.tile([128, NTILES], f32)    # -dbuf
        Abuf = singles.tile([128, NTILES], f32)
        Cbuf = singles.tile([128, NTILES], f32)
        rcb = singles.tile([128, NTILES], f32)
        stb = singles.tile([128, NTILES], f32)
        nthi = singles.tile([128, NTILES], f16)     # fp16(-tau)

        # scratch (single-buffered; same-engine ops serialize in order)
        act_scr = singles.tile([128, S], f16)
        dve_scr = singles.tile([128, S], f16)
        segC_scr = singles.tile([128, NSEG], f16)
        segA_scr = singles.tile([128, GROUP, NSEG], f16)
        segA_acc = singles.tile([128, GROUP, NSEG], f16)
        mrow_scr = singles.tile([128, NSEG], f16)

        zps_slots = PsumTag(psum, [128, 1024], f32, "zps", 2)

        def emit_proj(xpool):
            xq = xpool.tile([128, 2, S], f16, tag="xq")
            xk = xpool.tile([128, 2, S], f16, tag="xk")
            xv = xpool.tile([128, 2, S], f16, tag="xv")
            wq = xpool.tile([128, 2, 2 * HR], f16, tag="wq")
            wk = xpool.tile([128, 2, 2 * HR], f16, tag="wk")
            wv = xpool.tile([128, 2, 64], f16, tag="wv")
            d_wk = nc.sync.dma_start(wk[:], wkT_d[:].rearrange("(c p) d -> p c d", p=128))
            d_wq = nc.sync.dma_start(wq[:], wqT_d[:].rearrange("(c p) d -> p c d", p=128))
            xk_r = xkT_d[:].rearrange("(c p) i -> p c i", p=128)
            xq_r = xqT_d[:].rearrange("(c p) i -> p c i", p=128)
            d_xk, d_xq = [], []
            for n in (0, 1):
                sl = slice(n * 1024, (n + 1) * 1024)
                d_xk.append(nc.sync.dma_start(xk[:, :, sl], xk_r[:, :, sl]))
                d_xq.append(nc.sync.dma_start(xq[:, :, sl], xq_r[:, :, sl]))
            d_wv = nc.sync.dma_start(wv[:], wvT_d[:].rearrange("(c p) d -> p c d", p=128))
            d_xv = nc.sync.dma_start(xv[:], xvT_d[:].rearrange("(c p) i -> p c i", p=128))
            gates = {"k0": pe_fence([i_gw, d_wk, d_xk[0], i_bk, i_ones]),
                     "k1": pe_fence([d_xk[1]]),
                     "q0": pe_fence([d_wq, d_xq[0], i_bq]),
                     "q1": pe_fence([d_xq[1]]),
                     "v": pe_fence([d_wv, d_xv, i_vones])}

            qk_eps, v_eps = [], []
            # qT/kT: psum [66, 512] = W.T @ x per 512-col chunk, both heads
            # packed; aug rows come from the bias matmul (bk aug slot = 1).
            def emit_chunk(which, w, x, bias, dst, n):
                ps, gq, kq = zps_slots.tile([2 * HR, 512])
                sl = slice(n * 512, (n + 1) * 512)
                for c in range(2):
                    mm = nc.tensor.matmul(ps[:], w[:, c, :], x[:, c, sl],
                                          start=(c == 0), stop=False)
                    add_dep_helper(mm.ins, (gq or gates[which]).ins,
                                   sync=False, reason="ord")
                mm = nc.tensor.matmul(ps[:], bias[:], ones_row[:],
                                      start=False, stop=True)
                add_dep_helper(mm.ins, (gq or gates[which]).ins,
                               sync=False, reason="ord")
                e = nc.scalar.copy(dst[:, sl], ps[:])
                zps_slots.readers(kq, [e])
                return e

            for n in range(4):
                qk_eps.append(emit_chunk("k", wk, xk, bk_sb, kTa, n))
            q_eps_by_chunk = {0: emit_chunk("q", wq, xq, bq_sb, qTa, 0),
                              1: emit_chunk("q", wq, xq, bq_sb, qTa, 1)}

            def emit_late_q():
                q_eps_by_chunk[2] = emit_chunk("q", wq, xq, bq_sb, qTa, 2)
                q_eps_by_chunk[3] = emit_chunk("q", wq, xq, bq_sb, qTa, 3)

            # v[j, d] = x @ Wv.T (no bias; host adds it after normalize) --
            # deferred: the caller emits it off the z-matmul critical path,
            # on the ztp psum slots (idle until the first P stage)
            def emit_v(ztp_slots):
                for jt in range(NT):
                    ps_v, gv, kv = ztp_slots.tile([128, 64])
                    jsl = slice(jt * 128, (jt + 1) * 128)
                    for c in range(2):
                        mm = nc.tensor.matmul(ps_v[:], xv[:, c, jsl],
                                              wv[:, c, :],
                                              start=(c == 0), stop=(c == 1))
                        add_dep_helper(mm.ins, (gv or gates["v"]).ins,
                                       sync=False, reason="ord")
                    ev = nc.vector.tensor_copy(v_sb[:, jt, :, 0:32], ps_v[:])
                    ztp_slots.readers(kv, [ev])
                    v_eps.append(ev)
            return qk_eps, q_eps_by_chunk, emit_late_q, v_eps, emit_v

        def emit_main(k_eps, q_eps_by_chunk, emit_late_q, v_eps, emit_v):
            v_gate_deps = v_eps  # filled by emit_v, consumed by first stage_P
            once_deps = [i_ident]
            kgates, qgates = {}, {}

            def kgate_for(c):
                if c not in kgates:
                    kgates[c] = pe_fence([k_eps[c]])
                return kgates[c]

            def qgate_for(qc):
                if qc not in qgates:
                    qgates[qc] = pe_fence([q_eps_by_chunk[qc]])
                return qgates[qc]
            ztp_slots = PsumTag(ztpsum, [128, 512], f32, "ztp", 2)
            pv_slots = PsumTag(pvpsum, [HR, 512], f32, "pv", 2)

            zp_tiles = {}

            def tile_hd(t):
                return t // NT, t % NT  # head, i-tile

            def batched_update(gsl, lam):
                """dbuf += lam * (Abuf - 1) / Cbuf; ndbuf = -dbuf.
                C >= 1 is guaranteed while d < rowmax' (the max element always
                counts)."""
                nc.vector.reciprocal(rcb[:, gsl], Cbuf[:, gsl])
                nc.vector.scalar_tensor_tensor(stb[:, gsl], Abuf[:, gsl], -1.0,
                                               rcb[:, gsl], OP.add, OP.mult)
                nc.vector.scalar_tensor_tensor(dbuf[:, gsl], stb[:, gsl], lam,
                                               dbuf[:, gsl], OP.mult, OP.add)
                nc.vector.tensor_scalar(ndbuf[:, gsl], dbuf[:, gsl], -1.0,
                                        None, OP.mult)

            def stage_Z(grp):
                """z matmuls + relu copy to SBUF fp16 (ACT)."""
                g0 = grp * GROUP
                h = g0 // NT
                r0 = h * HR
                for t in range(g0, g0 + GROUP):
                    _, it = tile_hd(t)
                    isl = slice(it * 128, (it + 1) * 128)
                    zp = zpool.tile([128, S], f16, tag="zp")
                    for n in range(2):
                        zps, gz, kz = zps_slots.tile()
                        for m in range(2):
                            nsl = slice((2 * n + m) * 512, (2 * n + m + 1) * 512)
                            mm = nc.tensor.matmul(
                                zps[:, m * 512:(m + 1) * 512],
                                qTa[r0:r0 + 32, isl], kTa[r0:r0 + 32, nsl],
                                start=True, stop=True)
                            for dep in (gz, kgate_for(2 * n + m),
                                        qgate_for(it // 4)):
                                if dep is not None:
                                    add_dep_helper(mm.ins, dep.ins,
                                                   sync=False, reason="ord")
                        bsl = slice(n * 1024, (n + 1) * 1024)
                        cp = nc.scalar.activation(zp[:, bsl], zps[:], ACTF.Relu)
                        zps_slots.readers(kz, [cp])
                    zp_tiles[t] = zp

            def stage_T(grp):
                """fold tree to 256 strided groups + rowmax (DVE, fp16 2x)."""
                g0 = grp * GROUP
                for t in range(g0, g0 + GROUP):
                    zp = zp_tiles[t]
                    l1 = trpool.tile([128, 1024], f16, tag="l1")
                    l2 = trpool.tile([128, 512], f16, tag="l2")
                    l3 = l3pool.tile([128, NSEG], f16, tag="l3")
                    nc.vector.tensor_tensor(l1[:], zp[:, 0:1024],
                                            zp[:, 1024:2048], OP.max)
                    nc.vector.tensor_tensor(l2[:], l1[:, 0:512],
                                            l1[:, 512:1024], OP.max)
                    nc.vector.tensor_tensor(l3[:], l2[:, 0:256],
                                            l2[:, 256:512], OP.max)
                    nc.vector.tensor_scalar(
                        mrow_scr[:], l3[:], 0.0, None, OP.add, OP.max,
                        accum_out=mrow[:, t:t + 1])
                    zp_tiles[t] = (zp, l3)

            def stage_N_warm(grp, k):
                """one damped Newton iteration on group maxes (Pool+DVE)."""
                g0 = grp * GROUP
                gsl = slice(g0, g0 + GROUP)
                if k == 0:
                    # d0 = rowmax' - 1
                    nc.vector.tensor_scalar(dbuf[:, gsl], mrow[:, gsl], 1.0,
                                            None, OP.subtract)
                    nc.vector.tensor_scalar(ndbuf[:, gsl], mrow[:, gsl], -1.0,
                                            1.0, OP.mult, OP.add)
                for t in range(g0, g0 + GROUP, 2):
                    _, l3 = zp_tiles[t]
                    sl = t - g0
                    nc.gpsimd.tensor_scalar(
                        segA_scr[:, sl, :], l3[:], dbuf[:, t:t + 1], 0.0,
                        OP.subtract, OP.max)
                for t in range(g0, g0 + GROUP):
                    sl = t - g0
                    _, l3 = zp_tiles[t]
                    if t % 2 == 0:
                        nc.vector.tensor_scalar(
                            segA_acc[:, sl, :], segA_scr[:, sl, :], 0.0, None,
                            OP.add, OP.add, accum_out=Abuf[:, t:t + 1])
                    else:
                        nc.vector.tensor_scalar(
                            segA_acc[:, sl, :], l3[:], dbuf[:, t:t + 1], 0.0,
                            OP.subtract, OP.max)
                        nc.vector.tensor_scalar(
                            segA_acc[:, sl, :], segA_acc[:, sl, :], 0.0, None,
                            OP.add, OP.add, accum_out=Abuf[:, t:t + 1])
                    nc.vector.tensor_scalar(
                        segC_scr[:], l3[:], dbuf[:, t:t + 1], None,
                        OP.is_gt, OP.add, accum_out=Cbuf[:, t:t + 1])
                batched_update(gsl, WARM_LAMBDAS[k])

            def stage_N_newton(grp, it_n):
                """exact Newton: A on ACT (step 0) / DVE 2-op (step 1);
                C on DVE 4x."""
                g0 = grp * GROUP
                gsl = slice(g0, g0 + GROUP)
                for t in range(g0, g0 + GROUP):
                    zp, _ = zp_tiles[t]
                    if it_n == 0:
                        nc.scalar.activation(act_scr[:], zp[:], ACTF.Relu,
                                             bias=ndbuf[:, t:t + 1], scale=1.0,
                                             accum_out=Abuf[:, t:t + 1])
                    else:
                        # DVE 2-op form (both at 4x): relu scratch, then sum
                        nc.vector.tensor_scalar(
                            dve_scr[:], zp[:], dbuf[:, t:t + 1], 0.0,
                            OP.subtract, OP.max)
                        nc.vector.tensor_scalar(
                            dve_scr[:], dve_scr[:], 0.0, None,
                            OP.add, OP.add, accum_out=Abuf[:, t:t + 1])
                    nc.vector.tensor_scalar(dve_scr[:], zp[:],
                                            dbuf[:, t:t + 1], None,
                                            OP.is_gt, OP.add,
                                            accum_out=Cbuf[:, t:t + 1])
                batched_update(gsl, 1.0)
                if it_n == N_FULL - 1:
                    for t in range(g0, g0 + GROUP):
                        zp_tiles.pop(t)

            def stage_N_tau(grp):
                """finalize tau: fp16 cast, aug-row fill via PE transpose."""
                g0 = grp * GROUP
                gsl = slice(g0, g0 + GROUP)
                h = g0 // NT
                r0 = h * HR
                e_hi = nc.gpsimd.tensor_copy(nthi[:, gsl], ndbuf[:, gsl])

                # per tile: PE transpose [128,1] -> [1,128] -> qTa aug row
                pgate = pe_fence([e_hi] + once_deps)
                once_deps.clear()
                tau_eps = []
                for t in range(g0, g0 + GROUP):
                    _, it = tile_hd(t)
                    isl = slice(it * 128, (it + 1) * 128)
                    tps, gt, kt = ztp_slots.tile([1, 128], dtype=f16)
                    tr = nc.tensor.transpose(tps[:], nthi[:, t:t + 1], ident[:])
                    add_dep_helper(tr.ins, (gt or pgate).ins, sync=False,
                                   reason="ord")
                    ct = nc.vector.tensor_copy(qTa[r0 + 32:r0 + 33, isl], tps[:])
                    ztp_slots.readers(kt, [ct])
                    tau_eps.append(ct)
                return tau_eps

            def stage_P(grp, tau_eps):
                """zT (tau-shifted) + pT relu-copy + PV(+A) + copy-out."""
                g0 = grp * GROUP
                h = g0 // NT
                r0 = h * HR
                i0 = (g0 % NT) * 128
                NCK = GROUP * 128 // 512
                zgate = pe_fence(tau_eps + v_gate_deps)
                v_gate_deps.clear()
                pvs = [pv_slots.tile() for _ in range(NCK)]
                alt = 0
                # late groups: alternate zT chunks onto the (now idle) z-matmul
                # PSUM banks for a 4-deep copy pipeline in the drain phase
                borrow = grp >= NGRP - 2
                for jb in range(NT):
                    pT = ptpool.tile([128, GROUP * 128], f16, tag="pT")
                    jsl = slice(jb * 128, (jb + 1) * 128)
                    pcs = []
                    for cnk in range(NCK):
                        if borrow and (jb * NCK + cnk) % 2 == 1:
                            ztps, gzt, kzt0 = zps_slots.tile([128, 512])
                            kzt = ("z", kzt0)
                        else:
                            ztps, gzt, kzt0 = ztp_slots.tile()
                            kzt = ("t", kzt0)
                        csl = slice(i0 + cnk * 512, i0 + (cnk + 1) * 512)
                        mm = nc.tensor.matmul(ztps[:], kTa[r0:r0 + HR, jsl],
                                              qTa[r0:r0 + HR, csl],
                                              start=True, stop=True)
                        add_dep_helper(mm.ins, (gzt or zgate).ins, sync=False,
                                       reason="ord")
                        psl = slice(cnk * 512, (cnk + 1) * 512)
                        if alt % 4 < 3:
                            pc = nc.gpsimd.tensor_scalar(pT[:, psl], ztps[:],
                                                         0.0, None, OP.max)
                        else:
                            pc = nc.vector.tensor_scalar(pT[:, psl], ztps[:],
                                                         0.0, None, OP.max)
                        alt += 1
                        (ztp_slots if kzt[0] == "t" else zps_slots).readers(
                            kzt[1], [pc])
                        pcs.append(pc)
                    pgate2 = pe_fence(pcs)
                    for cnk in range(NCK):
                        pv_t, pv_g, pv_k = pvs[cnk]
                        mm = nc.tensor.matmul(pv_t[:], v_sb[:, jb, h, :],
                                              pT[:, cnk * 512:(cnk + 1) * 512],
                                              start=(jb == 0),
                                              stop=(jb == NT - 1))
                        add_dep_helper(mm.ins, pgate2.ins, sync=False,
                                       reason="ord")
                        if jb == 0 and pv_g is not None:
                            add_dep_helper(mm.ins, pv_g.ins, sync=False,
                                           reason="pv-slot")

                # copy-out (plain; host normalizes by the A row + adds bias)
                for cnk in range(NCK):
                    pv_t, pv_g, pv_k = pvs[cnk]
                    c0 = i0 + cnk * 512
                    oc = nc.gpsimd.tensor_copy(
                        outT_sb[r0:r0 + HR, c0:c0 + 512], pv_t[:])
                    pv_slots.readers(pv_k, [oc])

            # wavefront schedule: per-group step chains staggered by STRIDE
            # rows so engines always have ready work
            taus = {}
            gsteps = (["Z", "T"] + [f"w{k}" for k in range(len(WARM_LAMBDAS))]
                      + [f"n{k}" for k in range(N_FULL)] + ["tau", "P"])
            STRIDE = 3
            schedule = []
            nrows = (NGRP - 1) * STRIDE + len(gsteps)
            for r in range(nrows):
                for g in range(NGRP):
                    k = r - g * STRIDE
                    if 0 <= k < len(gsteps):
                        schedule.append((gsteps[k], g))
            z_seen = 0
            for op, g in schedule:
                if op == "Z":
                    stage_Z(g)
                    z_seen += 1
                    if z_seen == 1:
                        emit_late_q()
                    if z_seen == 2:
                        emit_v(ztp_slots)
                elif op == "T":
                    stage_T(g)
                elif op.startswith("w"):
                    stage_N_warm(g, int(op[1]))
                elif op.startswith("n"):
                    stage_N_newton(g, int(op[1]))
                elif op == "tau":
                    taus[g] = stage_N_tau(g)
                elif op == "P":
                    stage_P(g, taus.pop(g))

            nc.sync.dma_start(out_d[:], outT_sb[:])

        if loop_n > 1:
            xpool = ctx.enter_context(tc.tile_pool(name="xstage", bufs=1))
            zpool = ctx.enter_context(tc.tile_pool(name="zpool", bufs=24))
            trpool = ctx.enter_context(tc.tile_pool(name="trpool", bufs=2))
            l3pool = ctx.enter_context(tc.tile_pool(name="l3pool", bufs=24))
            ptpool = ctx.enter_context(tc.tile_pool(name="ptpool", bufs=6))
            with tc.For_i(0, loop_n, 1):
                emit_main(*emit_proj(xpool))
        else:
            xpool = ctx.enter_context(tc.tile_pool(name="xstage", bufs=1))
            proj_eps = emit_proj(xpool)
            zpool = ctx.enter_context(tc.tile_pool(name="zpool", bufs=24))
            trpool = ctx.enter_context(tc.tile_pool(name="trpool", bufs=2))
            l3pool = ctx.enter_context(tc.tile_pool(name="l3pool", bufs=24))
            ptpool = ctx.enter_context(tc.tile_pool(name="ptpool", bufs=6))
            emit_main(*proj_eps)

    nc.finalize()
    return nc


def _get_program(loop_n=1):
    global _PROGRAM
    if _PROGRAM is None:
        _PROGRAM = {}
    if loop_n not in _PROGRAM:
        _PROGRAM[loop_n] = _build_program(loop_n)
    return _PROGRAM[loop_n]


def _pad_w(w):
    """[64, 256] head-pair weight block -> padded [256, 66] fp16 (aug cols
    32/65 zero)."""
    out = np.zeros((D_MODEL, 2 * HR), np.float16)
    out[:, 0:32] = w[0:32].T.astype(np.float16)
    out[:, HR:HR + 32] = w[32:64].T.astype(np.float16)
    return np.ascontiguousarray(out)


def _pad_b(b, aug):
    out = np.zeros((1, 2 * HR), np.float16)
    out[0, 0:32] = b[0:32].astype(np.float16)
    out[0, HR:HR + 32] = b[32:64].astype(np.float16)
    out[0, 32] = aug
    out[0, HR + 32] = aug
    return np.ascontiguousarray(out)


def _make_in_maps(query, key, value, Wq, bq, Wk, bk, Wv, bv):
    """Host-side sharding: slicing/transposition + fp16 cast of x and W."""
    f, g = np.float32, np.float16
    xq = [np.ascontiguousarray(np.asarray(query, f)[b].T.astype(g)) for b in range(B)]
    xk = [np.ascontiguousarray(np.asarray(key, f)[b].T.astype(g)) for b in range(B)]
    xv = [np.ascontiguousarray(np.asarray(value, f)[b].T.astype(g)) for b in range(B)]
    in_maps = []
    for c in range(N_CORES):
        b, gidx = c // 4, c % 4
        dsl = slice(gidx * 64, (gidx + 1) * 64)
        in_maps.append({
            "xqT": xq[b],
            "xkT": xk[b],
            "xvT": xv[b],
            "wqT": _pad_w(np.asarray(Wq, f)[dsl] * SCALE),
            "wkT": _pad_w(np.asarray(Wk, f)[dsl]),
            "wvT": np.ascontiguousarray(np.asarray(Wv, f)[dsl].T.astype(g)),
            "bq": _pad_b(np.asarray(bq, f)[dsl] * SCALE, 0.0),
            "bk": _pad_b(np.asarray(bk, f)[dsl], 1.0),
        })
    return in_maps


def kernel(query, key, value, Wq, bq, Wk, bk, Wv, bv):
    from concourse.bass_utils import run_bass_kernel_spmd

    nc = _get_program()
    in_maps = _make_in_maps(query, key, value, Wq, bq, Wk, bk, Wv, bv)
    res = run_bass_kernel_spmd(nc, in_maps, list(range(N_CORES)))
    bv32 = np.asarray(bv, np.float32)
    out = np.empty((B, S, D_MODEL), np.float32)
    for c in range(N_CORES):
        b, gidx = c // 4, c % 4
        r = res.results[c]["out"]                     # [66, S]
        for h in range(2):
            pv = r[h * HR:h * HR + 32]                # [32, S]
            A = np.maximum(r[h * HR + 32], 1e-3)      # [S]
            cols = slice(gidx * 64 + h * 32, gidx * 64 + (h + 1) * 32)
            out[b, :, cols] = (pv / A[None, :]).T + bv32[cols][None, :]
    return out


# revision 6
# speedup vs baseline: 1.0154x; 1.0098x over previous
"""Trainium2 Bass kernel v3: MHA with sparsemax over the key dim.

Reference computation (B=2, S=2048, D=256, H=8, Dk=32):
    q = (query @ Wq.T + bq)  -> [B,S,H,Dk]   (k, v likewise)
    attn = einsum('bihd,bjhd->bijh', q, k) / sqrt(Dk)
    attn = sparsemax(attn, axis=-2)           # normalize over Sk (j) per (b,i,h)
    out  = einsum('bijh,bjhd->bihd', attn, v) -> reshape [B,S,256]

Sharding: 8 cores = 2 batches x 4 head-pairs. No collectives.

v3 design (vs v2): output NORMALIZATION replaces exact-tau convergence.
  The PV matmul's stationary v is augmented with a ones column, so PSUM row
  32 accumulates A_i = sum_j p[i,j] for free (matmul cost is moving-cols
  only). The host divides by A and adds the v-bias afterwards; sparsemax's
  sum-to-1 constraint is then exact regardless of tau error, which lets us
  drop the trapezoid pass, the submax probe/shift, and the fp16 tau hi/lo
  split (algo-sim rel err 4.9e-3 vs 2e-2 budget; work/algo_sim3.py).

  Pipeline per 128-row z tile:
    1. PE: z = qT.T @ kT (fp16) -> PSUM; ACT: z' = relu(z) fp16 -> SBUF.
    2. DVE: pairwise-max fold tree on z' -> 256 group maxes (l3) + rowmax.
    3. 3 damped Newton iters on the group-max surrogate from d0 = rowmax-1
       (Pool computes the relu(l3-d) scratch, DVE accumulates at 4x).
    4. 2 exact Newton steps on z': A via DVE 2-op (6 of 8 tiles) or ACT
       relu-bias-accum (2 of 8, rebalancing); C via DVE is_gt-accum 4x.
  p in column layout via a SECOND PE matmul zT = kTa.T @ qTa with one
  augmented contraction row (ones in kTa; -tau fp16 in qTa, filled per tile
  via a PE transpose). pT = relu(zT_psum) copies run on ACT (DVE gets no
  2x from an fp32-PSUM source, so ACT's 0.833 ns/col wins); PV is
  v-stationary with the ones column giving A. Output [97, S] d-major
  (head h at rows 64h: 32 pv rows + 1 A row); the host transposes and
  normalizes.

Engine balance (TimelineSim): ACT ~184us (z/pT copies, proj, 2/8 A-passes,
copy-out), DVE ~186us (fold, warm, A/C passes, tau, v), Pool ~45us (warm
scratch), PE ~93us -- ACT/DVE both ~85% busy over the 217us makespan.
Schedule: 8 groups of 4 tiles, wavefront stride 2; Newton A-scratches are
kept OFF Pool (its 1.39ns/col + per-op launch put it on the critical
chain); the last group's pT copies alternate ACT/DVE to share the drain.

Projection packing: qTa/kTa are [97, S] (head h at partition base 64h:
32 proj rows + 1 aug row -- matmul stationary bases must be 0/32/64), so
each 512-col projection chunk is ONE [97,512] PSUM->SBUF copy. The kTa
aug row of ones comes free from the bias K=1 matmul (bk_pad aug slot =
1.0). xq/xk DMAs are split in column halves so projections start early.

PE wait discipline: walrus allows only ONE sync wait on a PE LDWEIGHTS
struct, so every PE matmul/transpose is kept to at most one semaphore wait:
multi-source waits are absorbed by chains of tiny real LDWEIGHTS "fence"
instructions (one semaphore each), and PSUM slot reacquisition is gated on
the slot's previous reader (PsumTag).
"""

import numpy as np
from contextlib import ExitStack

HEADS = 8
D_MODEL = 256
DK = 32
B = 2
S = 2048
SCALE = float(1.0 / np.float32(np.sqrt(DK)))
N_CORES = 8
NT = 16            # i-tiles per head (2048/128)
NTILES = 32        # z tiles per core (2 heads x 16)
GROUP = 8          # tiles per group (half a head)
NGRP = NTILES // GROUP
NSEG = 256         # fold-tree level-3 groups per row
WARM_LAMBDAS = (1.5, 1.0, 1.0)
N_FULL = 2         # exact Newton steps
HR = 33            # rows per head in qTa/kTa (32 proj + 1 aug) and out

_PROGRAM = None


def _build_program(loop_n=1):
    import concourse.bass as bass
    import concourse.mybir as mybir
    import concourse.tile as tile
    from concourse import bacc
    from concourse.tile import add_dep_helper
    from concourse.masks import make_identity

    f32 = mybir.dt.float32
    f16 = mybir.dt.float16
    AX = mybir.AxisListType
    OP = mybir.AluOpType
    ACTF = mybir.ActivationFunctionType

    nc = bacc.Bacc("TRN2", target_bir_lowering=False, debug=False)

    # Per-core inputs (host pre-sliced / pre-transposed / fp16-cast).
    xqT_d = nc.dram_tensor("xqT", [D_MODEL, S], f16, kind="ExternalInput")
    xkT_d = nc.dram_tensor("xkT", [D_MODEL, S], f16, kind="ExternalInput")
    xvT_d = nc.dram_tensor("xvT", [D_MODEL, S], f16, kind="ExternalInput")
    wqT_d = nc.dram_tensor("wqT", [D_MODEL, 2 * HR], f16, kind="ExternalInput")
    wkT_d = nc.dram_tensor("wkT", [D_MODEL, 2 * HR], f16, kind="ExternalInput")
    wvT_d = nc.dram_tensor("wvT", [D_MODEL, 64], f16, kind="ExternalInput")
    bq_d = nc.dram_tensor("bq", [1, 2 * HR], f16, kind="ExternalInput")
    bk_d = nc.dram_tensor("bk", [1, 2 * HR], f16, kind="ExternalInput")
    out_d = nc.dram_tensor("out", [2 * HR, S], f32, kind="ExternalOutput")

    import concourse.mybir as _mb

    gate_state = {"w": None}

    def pe_fence(dep_instrs):
        """Chain of tiny real PE LDWEIGHTS instructions that absorb waits so
        later PE matmuls carry at most one semaphore wait each."""
        groups = {}
        for d in dep_instrs:
            if d is None:
                continue
            eng = d.ins.engine
            key = ("dma", d.ins.name) if eng == _mb.EngineType.SP else eng
            groups.setdefault(key, []).append(d)
        last = None
        for key, ds in groups.items():
            g = nc.tensor.ldweights(weights=gate_state["w"][:1, :1])
            for d in ds:
                add_dep_helper(g.ins, d.ins, sync=True, reason="pe-fence")
            if last is not None:
                add_dep_helper(g.ins, last.ins, sync=False, reason="pe-fence-chain")
            last = g
        return last

    class PsumTag:
        """Psum slot allocator wrapper that gates each slot's reacquisition
        on its previous reader via a PE fence (keeps matmul waits <= 1)."""

        def __init__(self, pool, shape, dtype, tag, bufs):
            self.pool, self.shape, self.dtype, self.tag = pool, shape, dtype, tag
            self.bufs = bufs
            self.hist = [None] * bufs
            self.i = 0

        def tile(self, shape=None, extra_deps=(), dtype=None):
            k = self.i % self.bufs
            self.i += 1
            deps = list(extra_deps)
            if self.hist[k]:
                deps.extend(self.hist[k])
            gate = pe_fence(deps) if deps else None
            t = self.pool.tile(shape or self.shape, dtype or self.dtype,
                               tag=self.tag)
            return t, gate, k

        def readers(self, k, instrs):
            self.hist[k] = [i for i in instrs if i is not None]

    with tile.TileContext(nc) as tc, ExitStack() as ctx:
        singles = ctx.enter_context(tc.tile_pool(name="singles", bufs=1))
        psum = ctx.enter_context(tc.tile_pool(name="psum", bufs=2, space="PSUM"))
        ztpsum = ctx.enter_context(tc.tile_pool(name="ztpsum", bufs=2, space="PSUM"))
        pvpsum = ctx.enter_context(tc.tile_pool(name="pvpsum", bufs=2, space="PSUM"))

        # ---- constants / small persistent buffers ----
        gate_w = singles.tile([1, 8], f16)
        i_gw = nc.vector.memset(gate_w, 0.0)
        gate_state["w"] = gate_w
        # preload the Relu activation table during the DMA/proj prologue
        nc.scalar.activation(gate_w[:], gate_w[:], mybir.ActivationFunctionType.Relu)
        ident = singles.tile([128, 128], f16)
        nc.gpsimd.memset(ident, 0.0)
        i_ident = nc.gpsimd.affine_select(
            out=ident, in_=ident, compare_op=mybir.AluOpType.not_equal,
            fill=1.0, base=0, pattern=[[-1, 128]], channel_multiplier=1)

        bq_sb = singles.tile([1, 2 * HR], f16)
        bk_sb = singles.tile([1, 2 * HR], f16)
        ones_row = singles.tile([1, 512], f16)
        i_ones = nc.vector.memset(ones_row, 1.0)
        i_bq = nc.sync.dma_start(bq_sb[:], bq_d[:])
        i_bk = nc.sync.dma_start(bk_sb[:], bk_d[:])

        # per-head packed projections: per head 32 proj rows + 1 aug row
        qTa = singles.tile([2 * HR, S], f16)
        kTa = singles.tile([2 * HR, S], f16)
        v_sb = singles.tile([128, NT, 2, HR], f16)  # [j%128, j//128, h, d+ones]
        i_vones = nc.vector.memset(v_sb[:, :, :, 32], 1.0)
        outT_sb = singles.tile([2 * HR, S], f32)    # [h*(33)+d | A, i]

        # per-tile stat columns [128, NTILES]
        mrow = singles.tile([128, NTILES], f32)     # rowmax of z'
        dbuf = singles.tile([128, NTILES], f32)     # tau estimate
        ndbuf = singles.tile([128, NTILES], f32)    # -dbuf
        Abuf = singles.tile([128, NTILES], f32)
        Cbuf = singles.tile([128, NTILES], f32)
        rcb = singles.tile([128, NTILES], f32)
        stb = singles.tile([128, NTILES], f32)
        nthi = singles.tile([128, NTILES], f16)     # fp16(-tau)

        # scratch (single-buffered; same-engine ops serialize in order)
        act_scr = singles.tile([128, S], f16)
        dve_scr = singles.tile([128, S], f16)
        segC_scr = singles.tile([128, NSEG], f16)
        segA_scr = singles.tile([128, GROUP, NSEG], f16)
        segA_acc = singles.tile([128, GROUP, NSEG], f16)
        mrow_scr = singles.tile([128, NSEG], f16)

        zps_slots = PsumTag(psum, [128, 1024], f32, "zps", 2)

        def emit_proj(xpool):
            xq = xpool.tile([128, 2, S], f16, tag="xq")
            xk = xpool.tile([128, 2, S], f16, tag="xk")
            xv = xpool.tile([128, 2, S], f16, tag="xv")
            wq = xpool.tile([128, 2, 2 * HR], f16, tag="wq")
            wk = xpool.tile([128, 2, 2 * HR], f16, tag="wk")
            wv = xpool.tile([128, 2, 64], f16, tag="wv")
            d_wk = nc.sync.dma_start(wk[:], wkT_d[:].rearrange("(c p) d -> p c d", p=128))
            d_wq = nc.sync.dma_start(wq[:], wqT_d[:].rearrange("(c p) d -> p c d", p=128))
            xk_r = xkT_d[:].rearrange("(c p) i -> p c i", p=128)
            xq_r = xqT_d[:].rearrange("(c p) i -> p c i", p=128)
            NSPL = 2
            d_xk, d_xq = [], []
            w = 2048 // NSPL
            for n in range(NSPL):
                sl = slice(n * w, (n + 1) * w)
                d_xk.append(nc.sync.dma_start(xk[:, :, sl], xk_r[:, :, sl]))
                d_xq.append(nc.sync.dma_start(xq[:, :, sl], xq_r[:, :, sl]))
            d_wv = nc.sync.dma_start(wv[:], wvT_d[:].rearrange("(c p) d -> p c d", p=128))
            d_xv = nc.sync.dma_start(xv[:], xvT_d[:].rearrange("(c p) i -> p c i", p=128))
            gates = {"k0": pe_fence([i_gw, d_wk, d_xk[0], i_bk, i_ones]),
                     "q0": pe_fence([d_wq, d_xq[0], i_bq]),
                     "v": pe_fence([d_wv, d_xv, i_vones])}
            for n in range(1, NSPL):
                gates[f"k{n}"] = pe_fence([d_xk[n]])
                gates[f"q{n}"] = pe_fence([d_xq[n]])

            qk_eps, v_eps = [], []
            # qT/kT: psum [66, 512] = W.T @ x per 512-col chunk, both heads
            # packed; aug rows come from the bias matmul (bk aug slot = 1).
            def emit_chunk(which, w, x, bias, dst, n):
                ps, gq, kq = zps_slots.tile([2 * HR, 512])
                sl = slice(n * 512, (n + 1) * 512)
                for c in range(2):
                    mm = nc.tensor.matmul(ps[:], w[:, c, :], x[:, c, sl],
                                          start=(c == 0), stop=False)
                    add_dep_helper(mm.ins, (gq or gates[which]).ins,
                                   sync=False, reason="ord")
                mm = nc.tensor.matmul(ps[:], bias[:], ones_row[:],
                                      start=False, stop=True)
                add_dep_helper(mm.ins, (gq or gates[which]).ins,
                               sync=False, reason="ord")
                e = nc.scalar.copy(dst[:, sl], ps[:])
                zps_slots.readers(kq, [e])
                return e

            for n in range(4):
                qk_eps.append(emit_chunk("k", wk, xk, bk_sb, kTa, n))
            q_eps_by_chunk = {0: emit_chunk("q", wq, xq, bq_sb, qTa, 0),
                              1: emit_chunk("q", wq, xq, bq_sb, qTa, 1)}

            def emit_late_q():
                q_eps_by_chunk[2] = emit_chunk("q", wq, xq, bq_sb, qTa, 2)
                q_eps_by_chunk[3] = emit_chunk("q", wq, xq, bq_sb, qTa, 3)

            # v[j, d] = x @ Wv.T (no bias; host adds it after normalize) --
            # deferred: the caller emits it off the z-matmul critical path,
            # on the ztp psum slots (idle until the first P stage)
            def emit_v(ztp_slots):
                for jt in range(NT):
                    ps_v, gv, kv = ztp_slots.tile([128, 64])
                    jsl = slice(jt * 128, (jt + 1) * 128)
                    for c in range(2):
                        mm = nc.tensor.matmul(ps_v[:], xv[:, c, jsl],
                                              wv[:, c, :],
                                              start=(c == 0), stop=(c == 1))
                        add_dep_helper(mm.ins, (gv or gates["v"]).ins,
                                       sync=False, reason="ord")
                    ev = nc.vector.tensor_copy(v_sb[:, jt, :, 0:32], ps_v[:])
                    ztp_slots.readers(kv, [ev])
                    v_eps.append(ev)
            return qk_eps, q_eps_by_chunk, emit_late_q, v_eps, emit_v

        def emit_main(k_eps, q_eps_by_chunk, emit_late_q, v_eps, emit_v):
            v_gate_deps = v_eps  # filled by emit_v, consumed by first stage_P
            once_deps = [i_ident]
            kgates, qgates = {}, {}

            def kgate_for(c):
                if c not in kgates:
                    kgates[c] = pe_fence([k_eps[c]])
                return kgates[c]

            def qgate_for(qc):
                if qc not in qgates:
                    qgates[qc] = pe_fence([q_eps_by_chunk[qc]])
                return qgates[qc]
            ztp_slots = PsumTag(ztpsum, [128, 512], f32, "ztp", 2)
            pv_slots = PsumTag(pvpsum, [HR, 512], f32, "pv", 2)

            zp_tiles = {}

            def tile_hd(t):
                return t // NT, t % NT  # head, i-tile

            def batched_update(gsl, lam):
                """dbuf += lam * (Abuf - 1) / Cbuf; ndbuf = -dbuf.
                C >= 1 is guaranteed while d < rowmax' (the max element always
                counts)."""
                nc.vector.reciprocal(rcb[:, gsl], Cbuf[:, gsl])
                nc.vector.scalar_tensor_tensor(stb[:, gsl], Abuf[:, gsl], -1.0,
                                               rcb[:, gsl], OP.add, OP.mult)
                nc.vector.scalar_tensor_tensor(dbuf[:, gsl], stb[:, gsl], lam,
                                               dbuf[:, gsl], OP.mult, OP.add)
                nc.vector.tensor_scalar(ndbuf[:, gsl], dbuf[:, gsl], -1.0,
                                        None, OP.mult)

            def stage_Z(grp):
                """z matmuls + relu copy to SBUF fp16 (ACT)."""
                g0 = grp * GROUP
                h = g0 // NT
                r0 = h * HR
                for t in range(g0, g0 + GROUP):
                    _, it = tile_hd(t)
                    isl = slice(it * 128, (it + 1) * 128)
                    zp = zpool.tile([128, S], f16, tag="zp")
                    for n in range(2):
                        zps, gz, kz = zps_slots.tile()
                        for m in range(2):
                            nsl = slice((2 * n + m) * 512, (2 * n + m + 1) * 512)
                            mm = nc.tensor.matmul(
                                zps[:, m * 512:(m + 1) * 512],
                                qTa[r0:r0 + 32, isl], kTa[r0:r0 + 32, nsl],
                                start=True, stop=True)
                            for dep in (gz, kgate_for(2 * n + m),
                                        qgate_for(it // 4)):
                                if dep is not None:
                                    add_dep_helper(mm.ins, dep.ins,
                                                   sync=False, reason="ord")
                        bsl = slice(n * 1024, (n + 1) * 1024)
                        cp = nc.scalar.activation(zp[:, bsl], zps[:], ACTF.Relu)
                        zps_slots.readers(kz, [cp])
                    zp_tiles[t] = zp

            def stage_T(grp):
                """fold tree to 256 strided groups + rowmax (DVE, fp16 2x)."""
                g0 = grp * GROUP
                for t in range(g0, g0 + GROUP):
                    zp = zp_tiles[t]
                    l1 = trpool.tile([128, 1024], f16, tag="l1")
                    l2 = trpool.tile([128, 512], f16, tag="l2")
                    l3 = l3pool.tile([128, NSEG], f16, tag="l3")
                    nc.vector.tensor_tensor(l1[:], zp[:, 0:1024],
                                            zp[:, 1024:2048], OP.max)
                    nc.vector.tensor_tensor(l2[:], l1[:, 0:512],
                                            l1[:, 512:1024], OP.max)
                    nc.vector.tensor_tensor(l3[:], l2[:, 0:256],
                                            l2[:, 256:512], OP.max)
                    nc.vector.tensor_scalar(
                        mrow_scr[:], l3[:], 0.0, None, OP.add, OP.max,
                        accum_out=mrow[:, t:t + 1])
                    zp_tiles[t] = (zp, l3)

            def stage_N_warm(grp, k):
                """one damped Newton iteration on group maxes (Pool+DVE)."""
                g0 = grp * GROUP
                gsl = slice(g0, g0 + GROUP)
                if k == 0:
                    # d0 = rowmax' - 1
                    nc.vector.tensor_scalar(dbuf[:, gsl], mrow[:, gsl], 1.0,
                                            None, OP.subtract)
                    nc.vector.tensor_scalar(ndbuf[:, gsl], mrow[:, gsl], -1.0,
                                            1.0, OP.mult, OP.add)
                for t in range(g0, g0 + GROUP, 2):
                    _, l3 = zp_tiles[t]
                    sl = t - g0
                    nc.gpsimd.tensor_scalar(
                        segA_scr[:, sl, :], l3[:], dbuf[:, t:t + 1], 0.0,
                        OP.subtract, OP.max)
                for t in range(g0, g0 + GROUP):
                    sl = t - g0
                    _, l3 = zp_tiles[t]
                    if t % 2 == 0:
                        nc.vector.tensor_scalar(
                            segA_acc[:, sl, :], segA_scr[:, sl, :], 0.0, None,
                            OP.add, OP.add, accum_out=Abuf[:, t:t + 1])
                    else:
                        nc.vector.tensor_scalar(
                            segA_acc[:, sl, :], l3[:], dbuf[:, t:t + 1], 0.0,
                            OP.subtract, OP.max)
                        nc.vector.tensor_scalar(
                            segA_acc[:, sl, :], segA_acc[:, sl, :], 0.0, None,
                            OP.add, OP.add, accum_out=Abuf[:, t:t + 1])
                    nc.vector.tensor_scalar(
                        segC_scr[:], l3[:], dbuf[:, t:t + 1], None,
                        OP.is_gt, OP.add, accum_out=Cbuf[:, t:t + 1])
                batched_update(gsl, WARM_LAMBDAS[k])

            def stage_N_newton(grp, it_n):
                """exact Newton: A on ACT (step 0) / DVE 2-op (step 1);
                C on DVE 4x."""
                g0 = grp * GROUP
                gsl = slice(g0, g0 + GROUP)
                for t in range(g0, g0 + GROUP):
                    zp, _ = zp_tiles[t]
                    if it_n == 0:
                        nc.scalar.activation(act_scr[:], zp[:], ACTF.Relu,
                                             bias=ndbuf[:, t:t + 1], scale=1.0,
                                             accum_out=Abuf[:, t:t + 1])
                    else:
                        # DVE 2-op form (both at 4x): relu scratch, then sum
                        nc.vector.tensor_scalar(
                            dve_scr[:], zp[:], dbuf[:, t:t + 1], 0.0,
                            OP.subtract, OP.max)
                        nc.vector.tensor_scalar(
                            dve_scr[:], dve_scr[:], 0.0, None,
                            OP.add, OP.add, accum_out=Abuf[:, t:t + 1])
                    nc.vector.tensor_scalar(dve_scr[:], zp[:],
                                            dbuf[:, t:t + 1], None,
                                            OP.is_gt, OP.add,
                                            accum_out=Cbuf[:, t:t + 1])
                batched_update(gsl, 1.0)
                if it_n == N_FULL - 1:
                    for t in range(g0, g0 + GROUP):
                        zp_tiles.pop(t)

            def stage_N_tau(grp):
                """finalize tau: fp16 cast, aug-row fill via PE transpose."""
                g0 = grp * GROUP
                gsl = slice(g0, g0 + GROUP)
                h = g0 // NT
                r0 = h * HR
                e_hi = nc.vector.tensor_copy(nthi[:, gsl], ndbuf[:, gsl])

                # per tile: PE transpose [128,1] -> [1,128] -> qTa aug row
                pgate = pe_fence([e_hi] + once_deps)
                once_deps.clear()
                tau_eps = []
                for t in range(g0, g0 + GROUP):
                    _, it = tile_hd(t)
                    isl = slice(it * 128, (it + 1) * 128)
                    tps, gt, kt = ztp_slots.tile([1, 128], dtype=f16)
                    tr = nc.tensor.transpose(tps[:], nthi[:, t:t + 1], ident[:])
                    add_dep_helper(tr.ins, (gt or pgate).ins, sync=False,
                                   reason="ord")
                    ct = nc.vector.tensor_copy(qTa[r0 + 32:r0 + 33, isl], tps[:])
                    ztp_slots.readers(kt, [ct])
                    tau_eps.append(ct)
                return tau_eps

            def stage_P(grp, tau_eps):
                """zT (tau-shifted) + pT relu-copy + PV(+A) + copy-out."""
                g0 = grp * GROUP
                h = g0 // NT
                r0 = h * HR
                i0 = (g0 % NT) * 128
                NCK = GROUP * 128 // 512
                zgate = pe_fence(tau_eps + v_gate_deps)
                v_gate_deps.clear()
                pvs = [pv_slots.tile() for _ in range(NCK)]
                alt = 0
                # late groups: alternate zT chunks onto the (now idle) z-matmul
                # PSUM banks for a 4-deep copy pipeline in the drain phase
                borrow = grp >= NGRP - 2
                for jb in range(NT):
                    pT = ptpool.tile([128, GROUP * 128], f16, tag="pT")
                    jsl = slice(jb * 128, (jb + 1) * 128)
                    pcs = []
                    for cnk in range(NCK):
                        if borrow and (jb * NCK + cnk) % 2 == 1:
                            ztps, gzt, kzt0 = zps_slots.tile([128, 512])
                            kzt = ("z", kzt0)
                        else:
                            ztps, gzt, kzt0 = ztp_slots.tile()
                            kzt = ("t", kzt0)
                        csl = slice(i0 + cnk * 512, i0 + (cnk + 1) * 512)
                        mm = nc.tensor.matmul(ztps[:], kTa[r0:r0 + HR, jsl],
                                              qTa[r0:r0 + HR, csl],
                                              start=True, stop=True)
                        add_dep_helper(mm.ins, (gzt or zgate).ins, sync=False,
                                       reason="ord")
                        psl = slice(cnk * 512, (cnk + 1) * 512)
                        if alt % 4 < 3:
                            pc = nc.gpsimd.tensor_scalar(pT[:, psl], ztps[:],
                                                         0.0, None, OP.max)
                        else:
                            pc = nc.vector.tensor_scalar(pT[:, psl], ztps[:],
                                                         0.0, None, OP.max)
                        alt += 1
                        (ztp_slots if kzt[0] == "t" else zps_slots).readers(
                            kzt[1], [pc])
                        pcs.append(pc)
                    pgate2 = pe_fence(pcs)
                    for cnk in range(NCK):
                        pv_t, pv_g, pv_k = pvs[cnk]
                        mm = nc.tensor.matmul(pv_t[:], v_sb[:, jb, h, :],
                                              pT[:, cnk * 512:(cnk + 1) * 512],
                                              start=(jb == 0),
                                              stop=(jb == NT - 1))
                        add_dep_helper(mm.ins, pgate2.ins, sync=False,
                                       reason="ord")
                        if jb == 0 and pv_g is not None:
                            add_dep_helper(mm.ins, pv_g.ins, sync=False,
                                           reason="pv-slot")

                # copy-out (plain; host normalizes by the A row + adds bias)
                for cnk in range(NCK):
                    pv_t, pv_g, pv_k = pvs[cnk]
                    c0 = i0 + cnk * 512
                    oc = nc.gpsimd.tensor_copy(
                        outT_sb[r0:r0 + HR, c0:c0 + 512], pv_t[:])
                    pv_slots.readers(pv_k, [oc])

            # wavefront schedule: per-group step chains staggered by STRIDE
            # rows so engines always have ready work
            taus = {}
            gsteps = (["Z", "T"] + [f"w{k}" for k in range(len(WARM_LAMBDAS))]
                      + [f"n{k}" for k in range(N_FULL)] + ["tau", "P"])
            STRIDE = 3
            schedule = []
            nrows = (NGRP - 1) * STRIDE + len(gsteps)
            for r in range(nrows):
                for g in range(NGRP):
                    k = r - g * STRIDE
                    if 0 <= k < len(gsteps):
                        schedule.append((gsteps[k], g))
            z_seen = 0
            for op, g in schedule:
                if op == "Z":
                    stage_Z(g)
                    z_seen += 1
                    if z_seen == 1:
                        emit_late_q()
                    if z_seen == 2:
                        emit_v(ztp_slots)
                elif op == "T":
                    stage_T(g)
                elif op.startswith("w"):
                    stage_N_warm(g, int(op[1]))
                elif op.startswith("n"):
                    stage_N_newton(g, int(op[1]))
                elif op == "tau":
                    taus[g] = stage_N_tau(g)
                elif op == "P":
                    stage_P(g, taus.pop(g))

            nc.sync.dma_start(out_d[:], outT_sb[:])

        if loop_n > 1:
            xpool = ctx.enter_context(tc.tile_pool(name="xstage", bufs=1))
            zpool = ctx.enter_context(tc.tile_pool(name="zpool", bufs=24))
            trpool = ctx.enter_context(tc.tile_pool(name="trpool", bufs=2))
            l3pool = ctx.enter_context(tc.tile_pool(name="l3pool", bufs=24))
            ptpool = ctx.enter_context(tc.tile_pool(name="ptpool", bufs=6))
            with tc.For_i(0, loop_n, 1):
                emit_main(*emit_proj(xpool))
        else:
            xpool = ctx.enter_context(tc.tile_pool(name="xstage", bufs=1))
            proj_eps = emit_proj(xpool)
            zpool = ctx.enter_context(tc.tile_pool(name="zpool", bufs=24))
            trpool = ctx.enter_context(tc.tile_pool(name="trpool", bufs=2))
            l3pool = ctx.enter_context(tc.tile_pool(name="l3pool", bufs=24))
            ptpool = ctx.enter_context(tc.tile_pool(name="ptpool", bufs=6))
            emit_main(*proj_eps)

    nc.finalize()
    return nc


def _get_program(loop_n=1):
    global _PROGRAM
    if _PROGRAM is None:
        _PROGRAM = {}
    if loop_n not in _PROGRAM:
        _PROGRAM[loop_n] = _build_program(loop_n)
    return _PROGRAM[loop_n]


def _pad_w(w):
    """[64, 256] head-pair weight block -> padded [256, 66] fp16 (aug cols
    32/65 zero)."""
    out = np.zeros((D_MODEL, 2 * HR), np.float16)
    out[:, 0:32] = w[0:32].T.astype(np.float16)
    out[:, HR:HR + 32] = w[32:64].T.astype(np.float16)
    return np.ascontiguousarray(out)


def _pad_b(b, aug):
    out = np.zeros((1, 2 * HR), np.float16)
    out[0, 0:32] = b[0:32].astype(np.float16)
    out[0, HR:HR + 32] = b[32:64].astype(np.float16)
    out[0, 32] = aug
    out[0, HR + 32] = aug
    return np.ascontiguousarray(out)


def _make_in_maps(query, key, value, Wq, bq, Wk, bk, Wv, bv):
    """Host-side sharding: slicing/transposition + fp16 cast of x and W."""
    f, g = np.float32, np.float16
    xq = [np.ascontiguousarray(np.asarray(query, f)[b].T.astype(g)) for b in range(B)]
    xk = [np.ascontiguousarray(np.asarray(key, f)[b].T.astype(g)) for b in range(B)]
    xv = [np.ascontiguousarray(np.asarray(value, f)[b].T.astype(g)) for b in range(B)]
    in_maps = []
    for c in range(N_CORES):
        b, gidx = c // 4, c % 4
        dsl = slice(gidx * 64, (gidx + 1) * 64)
        in_maps.append({
            "xqT": xq[b],
            "xkT": xk[b],
            "xvT": xv[b],
            "wqT": _pad_w(np.asarray(Wq, f)[dsl] * SCALE),
            "wkT": _pad_w(np.asarray(Wk, f)[dsl]),
            "wvT": np.ascontiguousarray(np.asarray(Wv, f)[dsl].T.astype(g)),
            "bq": _pad_b(np.asarray(bq, f)[dsl] * SCALE, 0.0),
            "bk": _pad_b(np.asarray(bk, f)[dsl], 1.0),
        })
    return in_maps


def kernel(query, key, value, Wq, bq, Wk, bk, Wv, bv):
    from concourse.bass_utils import run_bass_kernel_spmd

    nc = _get_program()
    in_maps = _make_in_maps(query, key, value, Wq, bq, Wk, bk, Wv, bv)
    res = run_bass_kernel_spmd(nc, in_maps, list(range(N_CORES)))
    bv32 = np.asarray(bv, np.float32)
    out = np.empty((B, S, D_MODEL), np.float32)
    for c in range(N_CORES):
        b, gidx = c // 4, c % 4
        r = res.results[c]["out"]                     # [66, S]
        for h in range(2):
            pv = r[h * HR:h * HR + 32]                # [32, S]
            A = np.maximum(r[h * HR + 32], 1e-3)      # [S]
            cols = slice(gidx * 64 + h * 32, gidx * 64 + (h + 1) * 32)
            out[b, :, cols] = (pv / A[None, :]).T + bv32[cols][None, :]
    return out


# revision 7
# speedup vs baseline: 1.0328x; 1.0171x over previous
"""Trainium2 Bass kernel v3: MHA with sparsemax over the key dim.

Reference computation (B=2, S=2048, D=256, H=8, Dk=32):
    q = (query @ Wq.T + bq)  -> [B,S,H,Dk]   (k, v likewise)
    attn = einsum('bihd,bjhd->bijh', q, k) / sqrt(Dk)
    attn = sparsemax(attn, axis=-2)           # normalize over Sk (j) per (b,i,h)
    out  = einsum('bijh,bjhd->bihd', attn, v) -> reshape [B,S,256]

Sharding: 8 cores = 2 batches x 4 head-pairs. No collectives.

v3 design (vs v2): output NORMALIZATION replaces exact-tau convergence.
  The PV matmul's stationary v is augmented with a ones column, so PSUM row
  32 accumulates A_i = sum_j p[i,j] for free (matmul cost is moving-cols
  only). The host divides by A and adds the v-bias afterwards; sparsemax's
  sum-to-1 constraint is then exact regardless of tau error, which lets us
  drop the trapezoid pass, the submax probe/shift, and the fp16 tau hi/lo
  split (algo-sim rel err 4.9e-3 vs 2e-2 budget; work/algo_sim3.py).

  Pipeline per 128-row z tile:
    1. PE: z = qT.T @ kT (fp16) -> PSUM; ACT: z' = relu(z) fp16 -> SBUF.
    2. DVE: pairwise-max fold tree on z' -> 256 group maxes (l3) + rowmax.
    3. 3 damped Newton iters on the group-max surrogate from d0 = rowmax-1
       (Pool computes the relu(l3-d) scratch, DVE accumulates at 4x).
    4. 2 exact Newton steps on z': A via DVE 2-op (6 of 8 tiles) or ACT
       relu-bias-accum (2 of 8, rebalancing); C via DVE is_gt-accum 4x.
  p in column layout via a SECOND PE matmul zT = kTa.T @ qTa with one
  augmented contraction row (ones in kTa; -tau fp16 in qTa, filled per tile
  via a PE transpose). pT = relu(zT_psum) copies run on ACT (DVE gets no
  2x from an fp32-PSUM source, so ACT's 0.833 ns/col wins); PV is
  v-stationary with the ones column giving A. Output [97, S] d-major
  (head h at rows 64h: 32 pv rows + 1 A row); the host transposes and
  normalizes.

Engine balance (TimelineSim): ACT ~184us (z/pT copies, proj, 2/8 A-passes,
copy-out), DVE ~186us (fold, warm, A/C passes, tau, v), Pool ~45us (warm
scratch), PE ~93us -- ACT/DVE both ~85% busy over the 217us makespan.
Schedule: 8 groups of 4 tiles, wavefront stride 2; Newton A-scratches are
kept OFF Pool (its 1.39ns/col + per-op launch put it on the critical
chain); the last group's pT copies alternate ACT/DVE to share the drain.

Projection packing: qTa/kTa are [97, S] (head h at partition base 64h:
32 proj rows + 1 aug row -- matmul stationary bases must be 0/32/64), so
each 512-col projection chunk is ONE [97,512] PSUM->SBUF copy. The kTa
aug row of ones comes free from the bias K=1 matmul (bk_pad aug slot =
1.0). xq/xk DMAs are split in column halves so projections start early.

PE wait discipline: walrus allows only ONE sync wait on a PE LDWEIGHTS
struct, so every PE matmul/transpose is kept to at most one semaphore wait:
multi-source waits are absorbed by chains of tiny real LDWEIGHTS "fence"
instructions (one semaphore each), and PSUM slot reacquisition is gated on
the slot's previous reader (PsumTag).
"""

import numpy as np
from contextlib import ExitStack

HEADS = 8
D_MODEL = 256
DK = 32
B = 2
S = 2048
SCALE = float(1.0 / np.float32(np.sqrt(DK)))
N_CORES = 8
NT = 16            # i-tiles per head (2048/128)
NTILES = 32        # z tiles per core (2 heads x 16)
GROUP = 8          # tiles per group (half a head)
NGRP = NTILES // GROUP
NSEG = 256         # fold-tree level-3 groups per row
WARM_LAMBDAS = (1.5, 1.0, 1.0)
N_FULL = 2         # exact Newton steps
HR = 33            # rows per head in qTa/kTa (32 proj + 1 aug) and out

_PROGRAM = None


def _build_program(loop_n=1):
    import concourse.bass as bass
    import concourse.mybir as mybir
    import concourse.tile as tile
    from concourse import bacc
    from concourse.tile import add_dep_helper
    from concourse.masks import make_identity

    f32 = mybir.dt.float32
    f16 = mybir.dt.float16
    AX = mybir.AxisListType
    OP = mybir.AluOpType
    ACTF = mybir.ActivationFunctionType

    nc = bacc.Bacc("TRN2", target_bir_lowering=False, debug=False)

    # Per-core inputs (host pre-sliced / pre-transposed / fp16-cast).
    xqT_d = nc.dram_tensor("xqT", [D_MODEL, S], f16, kind="ExternalInput")
    xkT_d = nc.dram_tensor("xkT", [D_MODEL, S], f16, kind="ExternalInput")
    xvT_d = nc.dram_tensor("xvT", [D_MODEL, S], f16, kind="ExternalInput")
    wqT_d = nc.dram_tensor("wqT", [D_MODEL, 2 * HR], f16, kind="ExternalInput")
    wkT_d = nc.dram_tensor("wkT", [D_MODEL, 2 * HR], f16, kind="ExternalInput")
    wvT_d = nc.dram_tensor("wvT", [D_MODEL, 64], f16, kind="ExternalInput")
    bq_d = nc.dram_tensor("bq", [1, 2 * HR], f16, kind="ExternalInput")
    bk_d = nc.dram_tensor("bk", [1, 2 * HR], f16, kind="ExternalInput")
    out_d = nc.dram_tensor("out", [2 * HR, S], f32, kind="ExternalOutput")

    import concourse.mybir as _mb

    gate_state = {"w": None}

    def pe_fence(dep_instrs):
        """Chain of tiny real PE LDWEIGHTS instructions that absorb waits so
        later PE matmuls carry at most one semaphore wait each."""
        groups = {}
        for d in dep_instrs:
            if d is None:
                continue
            eng = d.ins.engine
            key = ("dma", d.ins.name) if eng == _mb.EngineType.SP else eng
            groups.setdefault(key, []).append(d)
        last = None
        for key, ds in groups.items():
            g = nc.tensor.ldweights(weights=gate_state["w"][:1, :1])
            for d in ds:
                add_dep_helper(g.ins, d.ins, sync=True, reason="pe-fence")
            if last is not None:
                add_dep_helper(g.ins, last.ins, sync=False, reason="pe-fence-chain")
            last = g
        return last

    class PsumTag:
        """Psum slot allocator wrapper that gates each slot's reacquisition
        on its previous reader via a PE fence (keeps matmul waits <= 1)."""

        def __init__(self, pool, shape, dtype, tag, bufs):
            self.pool, self.shape, self.dtype, self.tag = pool, shape, dtype, tag
            self.bufs = bufs
            self.hist = [None] * bufs
            self.i = 0

        def tile(self, shape=None, extra_deps=(), dtype=None):
            k = self.i % self.bufs
            self.i += 1
            deps = list(extra_deps)
            if self.hist[k]:
                deps.extend(self.hist[k])
            gate = pe_fence(deps) if deps else None
            t = self.pool.tile(shape or self.shape, dtype or self.dtype,
                               tag=self.tag)
            return t, gate, k

        def readers(self, k, instrs):
            self.hist[k] = [i for i in instrs if i is not None]

    with tile.TileContext(nc) as tc, ExitStack() as ctx:
        singles = ctx.enter_context(tc.tile_pool(name="singles", bufs=1))
        psum = ctx.enter_context(tc.tile_pool(name="psum", bufs=2, space="PSUM"))
        ztpsum = ctx.enter_context(tc.tile_pool(name="ztpsum", bufs=2, space="PSUM"))
        pvpsum = ctx.enter_context(tc.tile_pool(name="pvpsum", bufs=2, space="PSUM"))

        # ---- constants / small persistent buffers ----
        gate_w = singles.tile([1, 8], f16)
        i_gw = nc.vector.memset(gate_w, 0.0)
        gate_state["w"] = gate_w
        # preload the Relu activation table during the DMA/proj prologue
        nc.scalar.activation(gate_w[:], gate_w[:], mybir.ActivationFunctionType.Relu)
        ident = singles.tile([128, 128], f16)
        nc.gpsimd.memset(ident, 0.0)
        i_ident = nc.gpsimd.affine_select(
            out=ident, in_=ident, compare_op=mybir.AluOpType.not_equal,
            fill=1.0, base=0, pattern=[[-1, 128]], channel_multiplier=1)

        bq_sb = singles.tile([1, 2 * HR], f16)
        bk_sb = singles.tile([1, 2 * HR], f16)
        ones_row = singles.tile([1, 512], f16)
        i_ones = nc.vector.memset(ones_row, 1.0)
        i_bq = nc.sync.dma_start(bq_sb[:], bq_d[:])
        i_bk = nc.sync.dma_start(bk_sb[:], bk_d[:])

        # per-head packed projections: per head 32 proj rows + 1 aug row
        qTa = singles.tile([2 * HR, S], f16)
        kTa = singles.tile([2 * HR, S], f16)
        v_sb = singles.tile([128, NT, 2, HR], f16)  # [j%128, j//128, h, d+ones]
        i_vones = nc.vector.memset(v_sb[:, :, :, 32], 1.0)
        outT_sb = singles.tile([2 * HR, S], f32)    # [h*(33)+d | A, i]

        # per-tile stat columns [128, NTILES]
        mrow = singles.tile([128, NTILES], f32)     # rowmax of z'
        dbuf = singles.tile([128, NTILES], f32)     # tau estimate
        ndbuf = singles.tile([128, NTILES], f32)    # -dbuf
        Abuf = singles.tile([128, NTILES], f32)
        Cbuf = singles.tile([128, NTILES], f32)
        rcb = singles.tile([128, NTILES], f32)
        stb = singles.tile([128, NTILES], f32)
        nthi = singles.tile([128, NTILES], f16)     # fp16(-tau)

        # scratch (single-buffered; same-engine ops serialize in order)
        act_scr = singles.tile([128, S], f16)
        dve_scr = singles.tile([128, S], f16)
        segC_scr = singles.tile([128, NSEG], f16)
        segA_scr = singles.tile([128, GROUP, NSEG], f16)
        segA_acc = singles.tile([128, GROUP, NSEG], f16)
        mrow_scr = singles.tile([128, NSEG], f16)

        zps_slots = PsumTag(psum, [128, 1024], f32, "zps", 2)

        def emit_proj(xpool):
            xq = xpool.tile([128, 2, S], f16, tag="xq")
            xk = xpool.tile([128, 2, S], f16, tag="xk")
            xv = xpool.tile([128, 2, S], f16, tag="xv")
            wq = xpool.tile([128, 2, 2 * HR], f16, tag="wq")
            wk = xpool.tile([128, 2, 2 * HR], f16, tag="wk")
            wv = xpool.tile([128, 2, 64], f16, tag="wv")
            d_wk = nc.sync.dma_start(wk[:], wkT_d[:].rearrange("(c p) d -> p c d", p=128))
            d_wq = nc.sync.dma_start(wq[:], wqT_d[:].rearrange("(c p) d -> p c d", p=128))
            xk_r = xkT_d[:].rearrange("(c p) i -> p c i", p=128)
            xq_r = xqT_d[:].rearrange("(c p) i -> p c i", p=128)
            NSPL = 2
            d_xk, d_xq = [], []
            w = 2048 // NSPL
            for n in range(NSPL):
                sl = slice(n * w, (n + 1) * w)
                d_xk.append(nc.sync.dma_start(xk[:, :, sl], xk_r[:, :, sl]))
                d_xq.append(nc.sync.dma_start(xq[:, :, sl], xq_r[:, :, sl]))
            d_wv = nc.sync.dma_start(wv[:], wvT_d[:].rearrange("(c p) d -> p c d", p=128))
            d_xv = nc.sync.dma_start(xv[:], xvT_d[:].rearrange("(c p) i -> p c i", p=128))
            gates = {"k0": pe_fence([i_gw, d_wk, d_xk[0], i_bk, i_ones]),
                     "q0": pe_fence([d_wq, d_xq[0], i_bq]),
                     "v": pe_fence([d_wv, d_xv, i_vones])}
            for n in range(1, NSPL):
                gates[f"k{n}"] = pe_fence([d_xk[n]])
                gates[f"q{n}"] = pe_fence([d_xq[n]])

            qk_eps, v_eps = [], []
            # qT/kT: psum [66, 512] = W.T @ x per 512-col chunk, both heads
            # packed; aug rows come from the bias matmul (bk aug slot = 1).
            def emit_chunk(which, w, x, bias, dst, n):
                ps, gq, kq = zps_slots.tile([2 * HR, 512])
                sl = slice(n * 512, (n + 1) * 512)
                for c in range(2):
                    mm = nc.tensor.matmul(ps[:], w[:, c, :], x[:, c, sl],
                                          start=(c == 0), stop=False)
                    add_dep_helper(mm.ins, (gq or gates[which]).ins,
                                   sync=False, reason="ord")
                mm = nc.tensor.matmul(ps[:], bias[:], ones_row[:],
                                      start=False, stop=True)
                add_dep_helper(mm.ins, (gq or gates[which]).ins,
                               sync=False, reason="ord")
                e = nc.scalar.copy(dst[:, sl], ps[:])
                zps_slots.readers(kq, [e])
                return e

            for n in range(4):
                qk_eps.append(emit_chunk("k", wk, xk, bk_sb, kTa, n))
            q_eps_by_chunk = {0: emit_chunk("q", wq, xq, bq_sb, qTa, 0),
                              1: emit_chunk("q", wq, xq, bq_sb, qTa, 1)}

            def emit_late_q():
                q_eps_by_chunk[2] = emit_chunk("q", wq, xq, bq_sb, qTa, 2)
                q_eps_by_chunk[3] = emit_chunk("q", wq, xq, bq_sb, qTa, 3)

            # v[j, d] = x @ Wv.T (no bias; host adds it after normalize) --
            # deferred: the caller emits it off the z-matmul critical path,
            # on the ztp psum slots (idle until the first P stage)
            def emit_v(ztp_slots):
                for jt in range(NT):
                    ps_v, gv, kv = ztp_slots.tile([128, 64])
                    jsl = slice(jt * 128, (jt + 1) * 128)
                    for c in range(2):
                        mm = nc.tensor.matmul(ps_v[:], xv[:, c, jsl],
                                              wv[:, c, :],
                                              start=(c == 0), stop=(c == 1))
                        add_dep_helper(mm.ins, (gv or gates["v"]).ins,
                                       sync=False, reason="ord")
                    ev = nc.vector.tensor_copy(v_sb[:, jt, :, 0:32], ps_v[:])
                    ztp_slots.readers(kv, [ev])
                    v_eps.append(ev)
            return qk_eps, q_eps_by_chunk, emit_late_q, v_eps, emit_v

        def emit_main(k_eps, q_eps_by_chunk, emit_late_q, v_eps, emit_v):
            v_gate_deps = v_eps  # filled by emit_v, consumed by first stage_P
            once_deps = [i_ident]
            kgates, qgates = {}, {}

            def kgate_for(c):
                if c not in kgates:
                    kgates[c] = pe_fence([k_eps[c]])
                return kgates[c]

            def qgate_for(qc):
                if qc not in qgates:
                    qgates[qc] = pe_fence([q_eps_by_chunk[qc]])
                return qgates[qc]
            ztp_slots = PsumTag(ztpsum, [128, 512], f32, "ztp", 2)
            pv_slots = PsumTag(pvpsum, [HR, 512], f32, "pv", 2)

            zp_tiles = {}

            def tile_hd(t):
                return t // NT, t % NT  # head, i-tile

            def batched_update(gsl, lam, need_nd=True):
                """dbuf += lam * (Abuf - 1) / Cbuf; ndbuf = -dbuf (only when
                a consumer needs it before the next update). C >= 1 is
                guaranteed while d < rowmax'."""
                nc.vector.reciprocal(rcb[:, gsl], Cbuf[:, gsl])
                nc.vector.scalar_tensor_tensor(stb[:, gsl], Abuf[:, gsl], -1.0,
                                               rcb[:, gsl], OP.add, OP.mult)
                nc.vector.scalar_tensor_tensor(dbuf[:, gsl], stb[:, gsl], lam,
                                               dbuf[:, gsl], OP.mult, OP.add)
                if need_nd:
                    nc.vector.tensor_scalar(ndbuf[:, gsl], dbuf[:, gsl], -1.0,
                                            None, OP.mult)

            def stage_Z(grp):
                """z matmuls + relu copy to SBUF fp16 (ACT)."""
                g0 = grp * GROUP
                h = g0 // NT
                r0 = h * HR
                for t in range(g0, g0 + GROUP):
                    _, it = tile_hd(t)
                    isl = slice(it * 128, (it + 1) * 128)
                    zp = zpool.tile([128, S], f16, tag="zp")
                    for n in range(2):
                        zps, gz, kz = zps_slots.tile()
                        for m in range(2):
                            nsl = slice((2 * n + m) * 512, (2 * n + m + 1) * 512)
                            mm = nc.tensor.matmul(
                                zps[:, m * 512:(m + 1) * 512],
                                qTa[r0:r0 + 32, isl], kTa[r0:r0 + 32, nsl],
                                start=True, stop=True)
                            for dep in (gz, kgate_for(2 * n + m),
                                        qgate_for(it // 4)):
                                if dep is not None:
                                    add_dep_helper(mm.ins, dep.ins,
                                                   sync=False, reason="ord")
                        bsl = slice(n * 1024, (n + 1) * 1024)
                        cp = nc.scalar.activation(zp[:, bsl], zps[:], ACTF.Relu)
                        zps_slots.readers(kz, [cp])
                    zp_tiles[t] = zp

            def stage_T(grp):
                """fold tree to 256 strided groups + rowmax (DVE, fp16 2x)."""
                g0 = grp * GROUP
                for t in range(g0, g0 + GROUP):
                    zp = zp_tiles[t]
                    l1 = trpool.tile([128, 1024], f16, tag="l1")
                    l2 = trpool.tile([128, 512], f16, tag="l2")
                    l3 = l3pool.tile([128, NSEG], f16, tag="l3")
                    nc.vector.tensor_tensor(l1[:], zp[:, 0:1024],
                                            zp[:, 1024:2048], OP.max)
                    nc.vector.tensor_tensor(l2[:], l1[:, 0:512],
                                            l1[:, 512:1024], OP.max)
                    nc.vector.tensor_tensor(l3[:], l2[:, 0:256],
                                            l2[:, 256:512], OP.max)
                    nc.vector.tensor_scalar(
                        mrow_scr[:], l3[:], 0.0, None, OP.add, OP.max,
                        accum_out=mrow[:, t:t + 1])
                    zp_tiles[t] = (zp, l3)

            def stage_N_warm(grp, k):
                """one damped Newton iteration on group maxes (Pool+DVE)."""
                g0 = grp * GROUP
                gsl = slice(g0, g0 + GROUP)
                if k == 0:
                    # d0 = rowmax' - 1
                    nc.vector.tensor_scalar(dbuf[:, gsl], mrow[:, gsl], 1.0,
                                            None, OP.subtract)
                    nc.vector.tensor_scalar(ndbuf[:, gsl], mrow[:, gsl], -1.0,
                                            1.0, OP.mult, OP.add)
                for t in range(g0, g0 + GROUP, 2):
                    _, l3 = zp_tiles[t]
                    sl = t - g0
                    nc.gpsimd.tensor_scalar(
                        segA_scr[:, sl, :], l3[:], dbuf[:, t:t + 1], 0.0,
                        OP.subtract, OP.max)
                for t in range(g0, g0 + GROUP):
                    sl = t - g0
                    _, l3 = zp_tiles[t]
                    if t % 2 == 0:
                        nc.vector.tensor_scalar(
                            segA_acc[:, sl, :], segA_scr[:, sl, :], 0.0, None,
                            OP.add, OP.add, accum_out=Abuf[:, t:t + 1])
                    else:
                        nc.vector.tensor_scalar(
                            segA_acc[:, sl, :], l3[:], dbuf[:, t:t + 1], 0.0,
                            OP.subtract, OP.max)
                        nc.vector.tensor_scalar(
                            segA_acc[:, sl, :], segA_acc[:, sl, :], 0.0, None,
                            OP.add, OP.add, accum_out=Abuf[:, t:t + 1])
                    nc.vector.tensor_scalar(
                        segC_scr[:], l3[:], dbuf[:, t:t + 1], None,
                        OP.is_gt, OP.add, accum_out=Cbuf[:, t:t + 1])
                batched_update(gsl, WARM_LAMBDAS[k],
                               need_nd=(not _nd_skip
                                        or k == len(WARM_LAMBDAS) - 1))

            def stage_N_newton(grp, it_n):
                """exact Newton: A on ACT (step 0) / DVE 2-op (step 1);
                C on DVE 4x."""
                g0 = grp * GROUP
                gsl = slice(g0, g0 + GROUP)
                for t in range(g0, g0 + GROUP):
                    zp, _ = zp_tiles[t]
                    if it_n == 0:
                        nc.scalar.activation(act_scr[:], zp[:], ACTF.Relu,
                                             bias=ndbuf[:, t:t + 1], scale=1.0,
                                             accum_out=Abuf[:, t:t + 1])
                    else:
                        # DVE 2-op form (both at 4x): relu scratch, then sum
                        nc.vector.tensor_scalar(
                            dve_scr[:], zp[:], dbuf[:, t:t + 1], 0.0,
                            OP.subtract, OP.max)
                        nc.vector.tensor_scalar(
                            dve_scr[:], dve_scr[:], 0.0, None,
                            OP.add, OP.add, accum_out=Abuf[:, t:t + 1])
                    nc.vector.tensor_scalar(dve_scr[:], zp[:],
                                            dbuf[:, t:t + 1], None,
                                            OP.is_gt, OP.add,
                                            accum_out=Cbuf[:, t:t + 1])
                batched_update(gsl, 1.0)
                if it_n == N_FULL - 1:
                    for t in range(g0, g0 + GROUP):
                        zp_tiles.pop(t)

            def stage_N_tau(grp):
                """finalize tau: fp16 cast, aug-row fill via PE transpose."""
                g0 = grp * GROUP
                gsl = slice(g0, g0 + GROUP)
                h = g0 // NT
                r0 = h * HR
                e_hi = nc.vector.tensor_copy(nthi[:, gsl], ndbuf[:, gsl])

                # per tile: PE transpose [128,1] -> [1,128] -> qTa aug row
                pgate = pe_fence([e_hi] + once_deps)
                once_deps.clear()
                tau_eps = []
                for t in range(g0, g0 + GROUP):
                    _, it = tile_hd(t)
                    isl = slice(it * 128, (it + 1) * 128)
                    tps, gt, kt = ztp_slots.tile([1, 128], dtype=f16)
                    tr = nc.tensor.transpose(tps[:], nthi[:, t:t + 1], ident[:])
                    add_dep_helper(tr.ins, (gt or pgate).ins, sync=False,
                                   reason="ord")
                    ct = nc.vector.tensor_copy(qTa[r0 + 32:r0 + 33, isl], tps[:])
                    ztp_slots.readers(kt, [ct])
                    tau_eps.append(ct)
                return tau_eps

            def stage_P(grp, tau_eps):
                """zT (tau-shifted) + pT relu-copy + PV(+A) + copy-out."""
                g0 = grp * GROUP
                h = g0 // NT
                r0 = h * HR
                i0 = (g0 % NT) * 128
                NCK = GROUP * 128 // 512
                zgate = pe_fence(tau_eps + v_gate_deps)
                v_gate_deps.clear()
                pvs = [pv_slots.tile() for _ in range(NCK)]
                alt = 0
                # late groups: alternate zT chunks onto the (now idle) z-matmul
                # PSUM banks for a 4-deep copy pipeline in the drain phase
                borrow = grp >= NGRP - 2
                for jb in range(NT):
                    pT = ptpool.tile([128, GROUP * 128], f16, tag="pT")
                    jsl = slice(jb * 128, (jb + 1) * 128)
                    pcs = []
                    for cnk in range(NCK):
                        if borrow and (jb * NCK + cnk) % 2 == 1:
                            ztps, gzt, kzt0 = zps_slots.tile([128, 512])
                            kzt = ("z", kzt0)
                        else:
                            ztps, gzt, kzt0 = ztp_slots.tile()
                            kzt = ("t", kzt0)
                        csl = slice(i0 + cnk * 512, i0 + (cnk + 1) * 512)
                        mm = nc.tensor.matmul(ztps[:], kTa[r0:r0 + HR, jsl],
                                              qTa[r0:r0 + HR, csl],
                                              start=True, stop=True)
                        add_dep_helper(mm.ins, (gzt or zgate).ins, sync=False,
                                       reason="ord")
                        psl = slice(cnk * 512, (cnk + 1) * 512)
                        if alt % 4 < 3:
                            pc = nc.gpsimd.tensor_scalar(pT[:, psl], ztps[:],
                                                         0.0, None, OP.max)
                        else:
                            pc = nc.vector.tensor_scalar(pT[:, psl], ztps[:],
                                                         0.0, None, OP.max)
                        alt += 1
                        (ztp_slots if kzt[0] == "t" else zps_slots).readers(
                            kzt[1], [pc])
                        pcs.append(pc)
                    pgate2 = pe_fence(pcs)
                    for cnk in range(NCK):
                        pv_t, pv_g, pv_k = pvs[cnk]
                        mm = nc.tensor.matmul(pv_t[:], v_sb[:, jb, h, :],
                                              pT[:, cnk * 512:(cnk + 1) * 512],
                                              start=(jb == 0),
                                              stop=(jb == NT - 1))
                        add_dep_helper(mm.ins, pgate2.ins, sync=False,
                                       reason="ord")
                        if jb == 0 and pv_g is not None:
                            add_dep_helper(mm.ins, pv_g.ins, sync=False,
                                           reason="pv-slot")

                # copy-out (plain; host normalizes by the A row + adds bias)
                for cnk in range(NCK):
                    pv_t, pv_g, pv_k = pvs[cnk]
                    c0 = i0 + cnk * 512
                    oc = nc.gpsimd.tensor_copy(
                        outT_sb[r0:r0 + HR, c0:c0 + 512], pv_t[:])
                    pv_slots.readers(pv_k, [oc])

            # wavefront schedule: per-group step chains staggered by STRIDE
            # rows so engines always have ready work
            taus = {}
            gsteps = (["Z", "T"] + [f"w{k}" for k in range(len(WARM_LAMBDAS))]
                      + [f"n{k}" for k in range(N_FULL)] + ["tau", "P"])
            STRIDE = 3
            schedule = []
            nrows = (NGRP - 1) * STRIDE + len(gsteps)
            for r in range(nrows):
                for g in range(NGRP):
                    k = r - g * STRIDE
                    if 0 <= k < len(gsteps):
                        schedule.append((gsteps[k], g))
            z_seen = 0
            for op, g in schedule:
                if op == "Z":
                    stage_Z(g)
                    z_seen += 1
                    if z_seen == 1:
                        emit_late_q()
                    if z_seen == 2:
                        emit_v(ztp_slots)
                elif op == "T":
                    stage_T(g)
                elif op.startswith("w"):
                    stage_N_warm(g, int(op[1]))
                elif op.startswith("n"):
                    stage_N_newton(g, int(op[1]))
                elif op == "tau":
                    taus[g] = stage_N_tau(g)
                elif op == "P":
                    stage_P(g, taus.pop(g))

            nc.sync.dma_start(out_d[:], outT_sb[:])

        if loop_n > 1:
            xpool = ctx.enter_context(tc.tile_pool(name="xstage", bufs=1))
            zpool = ctx.enter_context(tc.tile_pool(name="zpool", bufs=24))
            trpool = ctx.enter_context(tc.tile_pool(name="trpool", bufs=2))
            l3pool = ctx.enter_context(tc.tile_pool(name="l3pool", bufs=24))
            ptpool = ctx.enter_context(tc.tile_pool(name="ptpool", bufs=6))
            with tc.For_i(0, loop_n, 1):
                emit_main(*emit_proj(xpool))
        else:
            xpool = ctx.enter_context(tc.tile_pool(name="xstage", bufs=1))
            proj_eps = emit_proj(xpool)
            zpool = ctx.enter_context(tc.tile_pool(name="zpool", bufs=24))
            trpool = ctx.enter_context(tc.tile_pool(name="trpool", bufs=2))
            l3pool = ctx.enter_context(tc.tile_pool(name="l3pool", bufs=24))
            ptpool = ctx.enter_context(tc.tile_pool(name="ptpool", bufs=6))
            emit_main(*proj_eps)

    nc.finalize()
    return nc


def _get_program(loop_n=1):
    global _PROGRAM
    if _PROGRAM is None:
        _PROGRAM = {}
    if loop_n not in _PROGRAM:
        _PROGRAM[loop_n] = _build_program(loop_n)
    return _PROGRAM[loop_n]


def _pad_w(w):
    """[64, 256] head-pair weight block -> padded [256, 66] fp16 (aug cols
    32/65 zero)."""
    out = np.zeros((D_MODEL, 2 * HR), np.float16)
    out[:, 0:32] = w[0:32].T.astype(np.float16)
    out[:, HR:HR + 32] = w[32:64].T.astype(np.float16)
    return np.ascontiguousarray(out)


def _pad_b(b, aug):
    out = np.zeros((1, 2 * HR), np.float16)
    out[0, 0:32] = b[0:32].astype(np.float16)
    out[0, HR:HR + 32] = b[32:64].astype(np.float16)
    out[0, 32] = aug
    out[0, HR + 32] = aug
    return np.ascontiguousarray(out)


def _make_in_maps(query, key, value, Wq, bq, Wk, bk, Wv, bv):
    """Host-side sharding: slicing/transposition + fp16 cast of x and W."""
    f, g = np.float32, np.float16
    xq = [np.ascontiguousarray(np.asarray(query, f)[b].T.astype(g)) for b in range(B)]
    xk = [np.ascontiguousarray(np.asarray(key, f)[b].T.astype(g)) for b in range(B)]
    xv = [np.ascontiguousarray(np.asarray(value, f)[b].T.astype(g)) for b in range(B)]
    in_maps = []
    for c in range(N_CORES):
        b, gidx = c // 4, c % 4
        dsl = slice(gidx * 64, (gidx + 1) * 64)
        in_maps.append({
            "xqT": xq[b],
            "xkT": xk[b],
            "xvT": xv[b],
            "wqT": _pad_w(np.asarray(Wq, f)[dsl] * SCALE),
            "wkT": _pad_w(np.asarray(Wk, f)[dsl]),
            "wvT": np.ascontiguousarray(np.asarray(Wv, f)[dsl].T.astype(g)),
            "bq": _pad_b(np.asarray(bq, f)[dsl] * SCALE, 0.0),
            "bk": _pad_b(np.asarray(bk, f)[dsl], 1.0),
        })
    return in_maps


def kernel(query, key, value, Wq, bq, Wk, bk, Wv, bv):
    from concourse.bass_utils import run_bass_kernel_spmd

    nc = _get_program()
    in_maps = _make_in_maps(query, key, value, Wq, bq, Wk, bk, Wv, bv)
    res = run_bass_kernel_spmd(nc, in_maps, list(range(N_CORES)))
    bv32 = np.asarray(bv, np.float32)
    out = np.empty((B, S, D_MODEL), np.float32)
    for c in range(N_CORES):
        b, gidx = c // 4, c % 4
        r = res.results[c]["out"]                     # [66, S]
        for h in range(2):
            pv = r[h * HR:h * HR + 32]                # [32, S]
            A = np.maximum(r[h * HR + 32], 1e-3)      # [S]
            cols = slice(gidx * 64 + h * 32, gidx * 64 + (h + 1) * 32)
            out[b, :, cols] = (pv / A[None, :]).T + bv32[cols][None, :]
    return out
